# revision 29
# baseline (speedup 1.0000x reference)
"""Trainium2 Bass kernel for nn_Attention_aware_msa (sparse_attention).

Sharding: one attention head per NeuronCore (8 heads / 8 cores), per the
tensor-parallel hint.  All cross-core traffic is done with collectives:

  - 4 chunked ReduceScatters produce each rank's row-slice of the
    head-summed attention (for sim_round2).
  - An AllGather of the L2-normalized v^T (bf16) gives every core the full
    stacked Vn^T [1024, 2048]; Sum_h vn_h vn_h^T == Vn Vn^T, so each core
    computes only its row-slice of the head-summed gram matrices (sim/obj
    masks) locally.
  - An AllToAll delivers Vn^T[:, my_rows] as the stationary operand for
    that row-slice without any rank-dependent addressing.

Compute layout: QKV projections contract C=1024 on the PE with bf16
operands (x and W are shipped as bf16; fp32 PSUM accumulation), producing
q/k/v directly in transposed [d=128, n] layout; scores and attn@v run with
f32r / bf16 operands at full PE rate.  Scores fold all scaling into the operands:
k' columns are pre-scaled by 25*cs[m]/|k_m| (cls) / 25/|k_m| (reg), and
the 1/|q_n| row factor rides the ACT exp's per-partition scale AP.  exp is
computed without row-max subtraction (|logits| <= 25, safe in f32).
attn = 0.5*(exp_c/rowsum_c + exp_r/rowsum_r) in bf16; attn @ v is computed
transposed (out[d, n]) with V-natural stationary tiles and PE-transposed
attn as the moving operand.

Host marshals x^T and per-head weight slices in, and reassembles the four
outputs (out_cls, out_reg, sim_round2, obj_mask) from per-core pieces.
"""
import numpy as np

try:
    import jax
    jax.config.update("jax_compilation_cache_dir", "/tmp/bass_jax_cache")
    jax.config.update("jax_persistent_cache_min_compile_time_secs", 0.0)
except Exception:
    pass

import concourse.bacc as bacc
import concourse.tile as tile
from concourse import mybir
from concourse.bass_utils import run_bass_kernel_spmd
from concourse.masks import make_identity

F32 = mybir.dt.float32
F32R = mybir.dt.float32r
BF16 = mybir.dt.bfloat16

B, N, C, H = 1, 2048, 1024, 8
HD = C // H            # 128 head dim
NCORES = 8
SCALE = 25.0
SIM_THRESH = 0.75
CONF_SIM_THRESH = 0.99
NCH = N // 128         # 16 row chunks of 128
NGRP = 4               # chunk groups (512 rows each) -> 4 ReduceScatters
GROWS = N // NGRP      # 512 rows per group
RPG = GROWS // NCORES  # 64 rows per rank per group


def build_program():
    nc = bacc.Bacc("TRN2", target_bir_lowering=False, debug=False,
                   num_devices=NCORES)

    # ---- parameters ----
    xt_cls = nc.declare_dram_parameter("xt_cls", [C, N], BF16, isOutput=False)
    xt_reg = nc.declare_dram_parameter("xt_reg", [C, N], BF16, isOutput=False)
    wt_cls = nc.declare_dram_parameter("wt_cls", [C, 3 * HD], BF16, isOutput=False)
    wt_reg = nc.declare_dram_parameter("wt_reg", [C, 3 * HD], BF16, isOutput=False)
    cs25 = nc.declare_dram_parameter("cs25", [1, N], F32, isOutput=False)

    avt_cls_o = nc.declare_dram_parameter("avt_cls", [HD, N], F32, isOutput=True)
    avt_reg_o = nc.declare_dram_parameter("avt_reg", [HD, N], F32, isOutput=True)
    vt_cls_o = nc.declare_dram_parameter("vt_cls", [HD, N], F32, isOutput=True)
    vt_reg_o = nc.declare_dram_parameter("vt_reg", [HD, N], F32, isOutput=True)
    sim_o = nc.declare_dram_parameter("sim_rows", [N // NCORES, N], F32, isOutput=True)
    obj_o = nc.declare_dram_parameter("obj_rows", [N // NCORES, N], F32, isOutput=True)

    rg = [list(range(NCORES))]
    AF = mybir.ActivationFunctionType
    OP = mybir.AluOpType
    X = mybir.AxisListType.X

    with tile.TileContext(nc) as tc:
        with (
            tc.tile_pool(name="pc", bufs=1) as pc,
            tc.tile_pool(name="dram", bufs=1, space="DRAM") as dram,
        ):
            # ---------- static tiles ----------
            ident_bf = pc.tile([128, 128], BF16, tag="ident_bf")
            make_identity(nc, ident_bf)
            ident_f32 = pc.tile([128, 128], F32, tag="ident_f32")
            make_identity(nc, ident_f32)
            ones_f = pc.tile([128, 1], F32, tag="ones_f")
            nc.vector.memset(ones_f, 1.0)
            ones_col = pc.tile([128, 1], F32R, tag="ones")
            nc.vector.tensor_copy(ones_col[:], ones_f[:])
            cs_t = pc.tile([1, N], F32, tag="cs")
            nc.sync.dma_start(out=cs_t, in_=cs25[:, :])

            # long-lived per-head tensors
            qt = {}    # f32r  q^T (unnormalized)
            kp = {}    # f32r  scaled k'^T
            vn = {}    # bf16  normalized v^T
            vnat = {}  # bf16  V natural, block j at [:, j*128:(j+1)*128]
            rq = {}    # [128, 16] f32 per-partition 1/|q|
            for nm in ("cls", "reg"):
                qt[nm] = pc.tile([128, N], F32R, tag=f"qt_{nm}", name=f"qt_{nm}")
                kp[nm] = pc.tile([128, N], F32R, tag=f"kp_{nm}", name=f"kp_{nm}")
                vn[nm] = pc.tile([128, N], BF16, tag=f"vn_{nm}", name=f"vn_{nm}")
                vnat[nm] = pc.tile([128, N], BF16, tag=f"vnat_{nm}", name=f"vnat_{nm}")
                rq[nm] = pc.tile([128, 16], F32, tag=f"rq_{nm}", name=f"rq_{nm}")

            rq_scratch = dram.tile([2, N], F32, name="rq_scratch")

            # ================= phase A+B: projections & norms =============
            with (
                tc.tile_pool(name="early", bufs=1) as early,
                tc.tile_pool(name="xt", bufs=3) as xtp,
                tc.tile_pool(name="sqp", bufs=2) as sqp,
                tc.tile_pool(name="ps_a", bufs=1, space="PSUM") as ps_a,
            ):
                for ii, nm in enumerate(("cls", "reg")):
                    xpar = xt_cls if nm == "cls" else xt_reg
                    wpar = wt_cls if nm == "cls" else wt_reg
                    wts = early.tile([128, 8, 3 * HD], BF16, tag=f"wt_{nm}",
                                     name=f"wt_{nm}")
                    nc.sync.dma_start(
                        out=wts, in_=wpar.ap().rearrange("(c8 p) d -> p c8 d",
                                                         p=128))
                    vt = early.tile([128, N], F32, tag=f"vt_{nm}", name=f"vt_{nm}")
                    ssq = {}
                    for t in ("q", "k", "v"):
                        ssq[t] = early.tile([1, N], F32, tag=f"ssq_{nm}_{t}",
                                            name=f"ssq_{nm}_{t}")
                    dst = {"q": qt[nm], "k": kp[nm], "v": vt}
                    for half in range(2):
                        pps = {}
                        for pi in range(3):
                            for nl in range(2):
                                pps[pi, nl] = ps_a.tile([128, 512], F32, tag=f"proj{pi}{nl}", name=f"pps{pi}{nl}")
                        for c8 in range(8):
                            xc = xtp.tile([128, 1024], BF16, tag="xchunk")
                            nc.sync.dma_start(
                                out=xc,
                                in_=xpar[c8 * 128:(c8 + 1) * 128,
                                         half * 1024:(half + 1) * 1024])
                            for nl in range(2):
                                for pi in range(3):
                                    nc.tensor.matmul(
                                        pps[pi, nl][:],
                                        wts[:, c8, pi * HD:(pi + 1) * HD],
                                        xc[:, nl * 512:(nl + 1) * 512],
                                        start=(c8 == 0), stop=(c8 == 7))
                        for pi, t in enumerate(("q", "k", "v")):
                            for nl in range(2):
                                n4 = half * 2 + nl
                                sl = slice(n4 * 512, (n4 + 1) * 512)
                                sq = sqp.tile([128, 512], F32R, tag=f"sq_{nm}")
                                nc.scalar.activation(out=sq[:], in_=pps[pi, nl][:],
                                                     func=AF.Square)
                                nc.any.tensor_copy(out=dst[t][:, sl],
                                                   in_=pps[pi, nl][:])
                                sps = ps_a.tile([1, 512], F32, tag="misc_ps", bufs=2)
                                nc.tensor.matmul(sps[:], ones_col[:], sq[:],
                                                 start=True, stop=True)
                                nc.any.tensor_copy(out=ssq[t][:, sl],
                                                   in_=sps[:])

                    # ---------- norms for this input ----------
                    # k' = k * (25*cs/|k|) [cls]  or  k * (25/|k|) [reg]
                    rk = early.tile([1, N], F32, tag=f"rowa_{nm}", name="rk")
                    nc.scalar.activation(out=rk[:], in_=ssq["k"][:], func=AF.Sqrt)
                    nc.vector.reciprocal(rk[:], rk[:])
                    bk = early.tile([1, N], F32, tag=f"rowb_{nm}", name="bk")
                    if nm == "cls":
                        nc.vector.tensor_tensor(out=bk[:], in0=rk[:], in1=cs_t[:],
                                                op=OP.mult)
                    else:
                        nc.vector.tensor_scalar_mul(out=bk[:], in0=rk[:],
                                                    scalar1=SCALE)
                    bk_bc = early.tile([128, N], F32, tag="bc_big", name="bk_bc")
                    last_bcast = nc.gpsimd.partition_broadcast(bk_bc[:], bk[:])
                    nc.vector.tensor_tensor(out=kp[nm][:],
                                            in0=kp[nm][:].bitcast(F32),
                                            in1=bk_bc[:], op=OP.mult)

                    # vn = v / |v| (bf16)
                    rv = early.tile([1, N], F32, tag=f"rowc_{nm}", name="rv")
                    nc.scalar.activation(out=rv[:], in_=ssq["v"][:], func=AF.Sqrt)
                    nc.vector.reciprocal(rv[:], rv[:])
                    rv_bc = early.tile([128, N], F32, tag="bc_big", name="rv_bc")
                    last_bcast = nc.gpsimd.partition_broadcast(rv_bc[:], rv[:])
                    nc.vector.tensor_tensor(out=vn[nm][:], in0=vt[:],
                                            in1=rv_bc[:], op=OP.mult)

                    # rq: 1/|q_n| rearranged to partition-major [128, 16]
                    nc.sync.dma_start(out=rq_scratch[ii], in_=ssq["q"][0:1, :])
                    nc.sync.dma_start(
                        out=rq[nm],
                        in_=rq_scratch[ii].rearrange("(j p) -> p j", p=128))
                    nc.scalar.activation(out=rq[nm][:], in_=rq[nm][:], func=AF.Sqrt)
                    nc.vector.reciprocal(rq[nm][:], rq[nm][:])

                    # V natural (bf16): PE transpose of raw v^T
                    for j in range(NCH):
                        tps = ps_a.tile([128, 128], F32, tag="misc_ps", bufs=2)
                        nc.tensor.transpose(
                            tps[:], vt[:, j * 128:(j + 1) * 128], ident_f32[:])
                        nc.any.tensor_copy(
                            out=vnat[nm][:, j * 128:(j + 1) * 128], in_=tps[:])

                    # raw v^T is a kernel output
                    nc.sync.dma_start(
                        out=(vt_cls_o if nm == "cls" else vt_reg_o)[:, :],
                        in_=vt[:])

            # ---------- collectives: AllGather + AllToAll of vn ----------
            # Keep the Pool queue clear for phase-B broadcasts: every
            # collective waits on the last partition_broadcast so the Tile
            # scheduler cannot hoist a long AllGather ahead of them.
            from concourse.bass import _add_dep_helper
            NMI = {"cls": 0, "reg": 1}
            # one merged AllGather: rank block = [nm, 128, N]
            agi = dram.tile([2, 128, N], BF16, name="agi")
            for nm in ("cls", "reg"):
                nc.sync.dma_start(out=agi[NMI[nm]], in_=vn[nm][:])
            ag_out = dram.tile([NCORES, 2, 128, N], BF16, name="ago",
                               addr_space="Shared")
            cc = nc.gpsimd.collective_compute(
                "AllGather", OP.bypass, replica_groups=rg,
                ins=[agi[:].opt()], outs=[ag_out[:].opt()])
            _add_dep_helper(cc.ins, last_bcast.ins, sync=True,
                            reason="collectives after phase-B broadcasts")

            # one merged AllToAll: dest block j = both nm's columns
            # {g*512 + j*64 .. +64 : g in 0..3}
            a2i = dram.tile([NCORES, 2, 128, NGRP, RPG], BF16, name="a2i")
            for nm in ("cls", "reg"):
                for j in range(NCORES):
                    for g in range(NGRP):
                        c0 = g * GROWS + j * RPG
                        nc.sync.dma_start(out=a2i[j, NMI[nm], :, g, :],
                                          in_=vn[nm][:, c0:c0 + RPG])
            a2a_out = dram.tile([NCORES, 2, 128, NGRP, RPG], BF16, name="a2o")
            cc = nc.gpsimd.collective_compute(
                "AllToAll", OP.bypass, replica_groups=rg,
                ins=[a2i[:].opt()], outs=[a2a_out[:].opt()])
            _add_dep_helper(cc.ins, last_bcast.ins, sync=True,
                            reason="collectives after phase-B broadcasts")

            # ================= phase C: attention =========================
            att_bounce = [dram.tile([GROWS, N], BF16, name=f"attb_{g}") for g in range(NGRP)]
            rs_out = [dram.tile([RPG, N], BF16, name=f"rsout_{g}") for g in range(NGRP)]

            with (
                tc.tile_pool(name="cw", bufs=1) as cw,
                tc.tile_pool(name="attn", bufs=9) as attnp,
                tc.tile_pool(name="attnT", bufs=2) as attnTp,
                tc.tile_pool(name="avsb", bufs=2) as avsb,
                tc.tile_pool(name="ps_c", bufs=3, space="PSUM") as ps_c,
                tc.tile_pool(name="ps_t", bufs=1, space="PSUM") as ps_t,
                tc.tile_pool(name="ps_av", bufs=1, space="PSUM") as ps_av,
            ):
                attn_chunks = []
                atw = None
                for i in range(NCH):
                    g = i // 4
                    if i % 4 == 0:
                        atw = attnTp.tile([128, NCH * 512], BF16, tag="attnT",
                                          name=f"atw_{g}")
                    ec = {}
                    rec = {}
                    for nm in ("cls", "reg"):
                        e = cw.tile([128, N], F32, tag=f"e_{nm}", name=f"e_{nm}", bufs=2)
                        parts = cw.tile([128, 2], F32, tag=f"parts_{nm}", name=f"parts_{nm}", bufs=3)
                        for mh in range(2):
                            # two-bank score psum: 2 matmuls, ONE exp over
                            # [128, 1024] (halves the per-chunk hop count)
                            sps = ps_c.tile([128, 1024], F32, tag="score")
                            for mq in range(2):
                                m4 = mh * 2 + mq
                                nc.tensor.matmul(
                                    sps[:, mq * 512:(mq + 1) * 512],
                                    qt[nm][:, i * 128:(i + 1) * 128],
                                    kp[nm][:, m4 * 512:(m4 + 1) * 512],
                                    start=True, stop=True)
                            nc.scalar.activation(
                                out=e[:, mh * 1024:(mh + 1) * 1024], in_=sps[:],
                                func=AF.Exp, scale=rq[nm][:, i:i + 1],
                                accum_out=parts[:, mh:mh + 1])
                        rs_sum = cw.tile([128, 1], F32, tag=f"rssum_{nm}", name=f"rssum_{nm}", bufs=3)
                        nc.vector.reduce_sum(out=rs_sum[:], in_=parts[:], axis=X)
                        rc = cw.tile([128, 1], F32, tag=f"rec_{nm}", name=f"rec_{nm}", bufs=3)
                        nc.vector.reciprocal(rc[:], rs_sum[:])
                        ec[nm] = e
                        rec[nm] = rc

                    t1 = cw.tile([128, N], F32, tag="t1")
                    nc.vector.tensor_scalar(
                        out=t1[:], in0=ec["cls"][:], scalar1=rec["cls"][:],
                        scalar2=0.5, op0=OP.mult, op1=OP.mult)
                    t2 = cw.tile([128, N], F32, tag="t2")
                    nc.vector.tensor_scalar(
                        out=t2[:], in0=ec["reg"][:], scalar1=rec["reg"][:],
                        scalar2=0.5, op0=OP.mult, op1=OP.mult)
                    ab = attnp.tile([128, N], BF16, tag="attn_bf")
                    nc.vector.tensor_tensor(out=ab[:], in0=t1[:], in1=t2[:],
                                            op=OP.add)
                    attn_chunks.append(ab)
                    nc.sync.dma_start(
                        out=att_bounce[g][(i % 4) * 128:(i % 4 + 1) * 128, :],
                        in_=ab[:])

                    if i % 4 == 3:
                        # ReduceScatter for this group of 512 rows
                        nc.gpsimd.collective_compute(
                            "ReduceScatter", OP.add, replica_groups=rg,
                            ins=[att_bounce[g][:].opt()],
                            outs=[rs_out[g][:].opt()])

                        # transpose the 4 chunks -> attnT window [m, 512]
                        for j in range(NCH):
                            tps = ps_t.tile([128, 512], BF16, tag="attr")
                            for ii in range(4):
                                nc.tensor.transpose(
                                    tps[:, ii * 128:(ii + 1) * 128],
                                    attn_chunks[g * 4 + ii][:, j * 128:(j + 1) * 128],
                                    ident_bf[:])
                            nc.any.tensor_copy(
                                out=atw[:, j * 512:(j + 1) * 512], in_=tps[:])

                        # AV for these 512 output columns
                        for nm, opar in (("cls", avt_cls_o), ("reg", avt_reg_o)):
                            aps = ps_av.tile([128, 512], F32, tag="av")
                            for j in range(NCH):
                                nc.tensor.matmul(
                                    aps[:],
                                    vnat[nm][:, j * 128:(j + 1) * 128],
                                    atw[:, j * 512:(j + 1) * 512],
                                    start=(j == 0), stop=(j == NCH - 1))
                            av_s = avsb.tile([128, 512], F32, tag="av_sb")
                            nc.any.tensor_copy(out=av_s[:], in_=aps[:])
                            nc.sync.dma_start(
                                out=opar[:, g * 512:(g + 1) * 512], in_=av_s[:])

            # ================= phase D: gram row-slices + masks ===========
            with (
                tc.tile_pool(name="late", bufs=1) as late,
                tc.tile_pool(name="vgp", bufs=3) as vgp,
                tc.tile_pool(name="pe_big", bufs=3) as peb,
                tc.tile_pool(name="ps_g", bufs=1, space="PSUM") as ps_g,
            ):
                masks = {}
                for nm, thr in (("cls", SIM_THRESH * H), ("reg", CONF_SIM_THRESH * H)):
                    a2s = late.tile([128, NCORES * NGRP * RPG], BF16, tag=f"a2s_{nm}", name=f"a2s_{nm}")
                    nc.sync.dma_start(
                        out=a2s[:].rearrange("p (h g r) -> p h g r",
                                             h=NCORES, g=NGRP),
                        in_=a2a_out[:, NMI[nm]].rearrange("h p g r -> p h g r"))
                    mask = late.tile([128, 2, N], F32, tag=f"mask_{nm}", name=f"mask_{nm}")
                    gt = {}
                    for mm in range(2):
                        for m4 in range(4):
                            gt[mm, m4] = ps_g.tile([128, 512], F32, tag=f"gram{mm}{m4}", name=f"gram{mm}{m4}")
                    for c8 in range(8):
                        vg = vgp.tile([128, N], BF16, tag="vg")
                        nc.sync.dma_start(out=vg, in_=ag_out[c8, NMI[nm]])
                        for mm in range(2):
                            for m4 in range(4):
                                nc.tensor.matmul(
                                    gt[mm, m4][:],
                                    a2s[:, c8 * 256 + mm * 128:
                                        c8 * 256 + (mm + 1) * 128],
                                    vg[:, m4 * 512:(m4 + 1) * 512],
                                    start=(c8 == 0), stop=(c8 == 7))
                    for mm in range(2):
                        for m4 in range(4):
                            nc.vector.tensor_scalar(
                                out=mask[:, mm, m4 * 512:(m4 + 1) * 512],
                                in0=gt[mm, m4][:], scalar1=float(thr),
                                scalar2=None, op0=OP.is_gt)
                    masks[nm] = mask

                # ---------- phase E: sim_round2 / obj_mask rows ----------
                for mm in range(2):
                    srows = late.tile([128, N], BF16, tag="srows")
                    for gg in range(2):
                        g = mm * 2 + gg
                        nc.sync.dma_start(out=srows[gg * RPG:(gg + 1) * RPG, :],
                                          in_=rs_out[g][:, :])
                    es = peb.tile([128, N], F32, tag="pe_big")
                    essum = late.tile([128, 1], F32, tag="essum")
                    nc.scalar.activation(out=es[:], in_=srows[:], func=AF.Exp,
                                         scale=1.0 / H, accum_out=essum[:])
                    # sim2 = es * mask_cls / rowsum(es * mask_cls); the
                    # pre-mask softmax denominator cancels in the renorm.
                    msk = peb.tile([128, N], F32, tag="pe_big")
                    nc.vector.tensor_tensor(out=msk[:], in0=es[:],
                                            in1=masks["cls"][:, mm, :], op=OP.mult)
                    mrow = late.tile([128, 1], F32, tag="mrow")
                    nc.vector.reduce_sum(out=mrow[:], in_=msk[:], axis=X)
                    rmrow = late.tile([128, 1], F32, tag="rmrow")
                    nc.vector.reciprocal(rmrow[:], mrow[:])
                    sim2 = peb.tile([128, N], F32, tag="pe_big")
                    nc.vector.tensor_scalar(out=sim2[:], in0=msk[:],
                                            scalar1=rmrow[:], scalar2=None,
                                            op0=OP.mult)
                    nc.sync.dma_start(out=sim_o[mm * 128:(mm + 1) * 128, :],
                                      in_=sim2[:])

                    omsk = peb.tile([128, N], F32, tag="pe_big")
                    nc.vector.tensor_tensor(out=omsk[:], in0=sim2[:],
                                            in1=masks["reg"][:, mm, :], op=OP.mult)
                    orow = late.tile([128, 1], F32, tag="orow")
                    nc.vector.reduce_sum(out=orow[:], in_=omsk[:], axis=X)
                    rorow = late.tile([128, 1], F32, tag="rorow")
                    nc.vector.reciprocal(rorow[:], orow[:])
                    obj2 = peb.tile([128, N], F32, tag="pe_big")
                    nc.vector.tensor_scalar(out=obj2[:], in0=omsk[:],
                                            scalar1=rorow[:], scalar2=None,
                                            op0=OP.mult)
                    nc.sync.dma_start(out=obj_o[mm * 128:(mm + 1) * 128, :],
                                      in_=obj2[:])

    nc.compile()
    return nc


_NC_CACHE = None


def _get_program():
    global _NC_CACHE
    if _NC_CACHE is None:
        _NC_CACHE = build_program()
    return _NC_CACHE


def make_in_maps(x_cls, x_reg, cls_score, W_qkv_cls, W_qkv_reg):
    import ml_dtypes
    bf = ml_dtypes.bfloat16
    xt_cls = np.ascontiguousarray(x_cls[0].T).astype(bf)   # [C, N]
    xt_reg = np.ascontiguousarray(x_reg[0].T).astype(bf)
    cs25 = (SCALE * cls_score).reshape(1, N).astype(np.float32)
    in_maps = []
    for h in range(NCORES):
        rows = np.r_[h * HD:(h + 1) * HD,
                     C + h * HD:C + (h + 1) * HD,
                     2 * C + h * HD:2 * C + (h + 1) * HD]
        in_maps.append({
            "xt_cls": xt_cls,
            "xt_reg": xt_reg,
            "wt_cls": np.ascontiguousarray(W_qkv_cls[rows].T).astype(bf),
            "wt_reg": np.ascontiguousarray(W_qkv_reg[rows].T).astype(bf),
            "cs25": cs25,
        })
    return in_maps


def assemble(results):
    """results: list over cores of dicts of np arrays -> full outputs."""
    out_cls = np.empty((1, N, 2 * C), np.float32)
    out_reg = np.empty((1, N, 2 * C), np.float32)
    sim = np.empty((N, N), np.float32)
    obj = np.empty((N, N), np.float32)
    for h in range(NCORES):
        r = results[h]
        out_cls[0, :, h * HD:(h + 1) * HD] = r["avt_cls"].T
        out_cls[0, :, C + h * HD:C + (h + 1) * HD] = r["vt_cls"].T
        out_reg[0, :, h * HD:(h + 1) * HD] = r["avt_reg"].T
        out_reg[0, :, C + h * HD:C + (h + 1) * HD] = r["vt_reg"].T
        # device row order for rank h: for g in 0..3: rows g*512+h*64 .. +64
        ridx = np.concatenate([
            np.arange(g * GROWS + h * RPG, g * GROWS + (h + 1) * RPG)
            for g in range(NGRP)])
        sim[ridx] = r["sim_rows"]
        obj[ridx] = r["obj_rows"]
    return out_cls, out_reg, sim, obj


def kernel(x_cls, x_reg, cls_score, fg_score, W_qkv_cls, W_qkv_reg):
    x_cls = np.asarray(x_cls, np.float32)
    x_reg = np.asarray(x_reg, np.float32)
    cls_score = np.asarray(cls_score, np.float32)
    W_qkv_cls = np.asarray(W_qkv_cls, np.float32)
    W_qkv_reg = np.asarray(W_qkv_reg, np.float32)

    nc = _get_program()
    in_maps = make_in_maps(x_cls, x_reg, cls_score, W_qkv_cls, W_qkv_reg)
    res = run_bass_kernel_spmd(nc, in_maps, core_ids=list(range(NCORES)))
    return assemble(res.results)


# revision 30
# speedup vs baseline: 1.0713x; 1.0713x over previous
"""Trainium2 Bass kernel for nn_Attention_aware_msa (sparse_attention).

Sharding: one attention head per NeuronCore (8 heads / 8 cores), per the
tensor-parallel hint.  All cross-core traffic is done with collectives:

  - 4 chunked ReduceScatters produce each rank's row-slice of the
    head-summed attention (for sim_round2).
  - An AllGather of the L2-normalized v^T (bf16) gives every core the full
    stacked Vn^T [1024, 2048]; Sum_h vn_h vn_h^T == Vn Vn^T, so each core
    computes only its row-slice of the head-summed gram matrices (sim/obj
    masks) locally.
  - An AllToAll delivers Vn^T[:, my_rows] as the stationary operand for
    that row-slice without any rank-dependent addressing.

Compute layout: QKV projections contract C=1024 on the PE with bf16
operands (x and W are shipped as bf16; fp32 PSUM accumulation), producing
q/k/v directly in transposed [d=128, n] layout; scores and attn@v run with
f32r / bf16 operands at full PE rate.  Scores fold all scaling into the operands:
k' columns are pre-scaled by 25*cs[m]/|k_m| (cls) / 25/|k_m| (reg), and
the 1/|q_n| row factor rides the ACT exp's per-partition scale AP.  exp is
computed without row-max subtraction (|logits| <= 25, safe in f32).
attn = 0.5*(exp_c/rowsum_c + exp_r/rowsum_r) in bf16; attn @ v is computed
transposed (out[d, n]) with V-natural stationary tiles and PE-transposed
attn as the moving operand.

Host marshals x^T and per-head weight slices in, and reassembles the four
outputs (out_cls, out_reg, sim_round2, obj_mask) from per-core pieces.
"""
import numpy as np

try:
    import jax
    jax.config.update("jax_compilation_cache_dir", "/tmp/bass_jax_cache")
    jax.config.update("jax_persistent_cache_min_compile_time_secs", 0.0)
except Exception:
    pass

import concourse.bacc as bacc
import concourse.tile as tile
from concourse import mybir
from concourse.bass_utils import run_bass_kernel_spmd
from concourse.masks import make_identity

F32 = mybir.dt.float32
F32R = mybir.dt.float32r
BF16 = mybir.dt.bfloat16

B, N, C, H = 1, 2048, 1024, 8
HD = C // H            # 128 head dim
NCORES = 8
SCALE = 25.0
SIM_THRESH = 0.75
CONF_SIM_THRESH = 0.99
NCH = N // 128         # 16 row chunks of 128
NGRP = 4               # chunk groups (512 rows each) -> 4 ReduceScatters
GROWS = N // NGRP      # 512 rows per group
RPG = GROWS // NCORES  # 64 rows per rank per group


def build_program():
    nc = bacc.Bacc("TRN2", target_bir_lowering=False, debug=False,
                   num_devices=NCORES)

    # ---- parameters ----
    xt_cls = nc.declare_dram_parameter("xt_cls", [C, N], BF16, isOutput=False)
    xt_reg = nc.declare_dram_parameter("xt_reg", [C, N], BF16, isOutput=False)
    wt_cls = nc.declare_dram_parameter("wt_cls", [C, 3 * HD], BF16, isOutput=False)
    wt_reg = nc.declare_dram_parameter("wt_reg", [C, 3 * HD], BF16, isOutput=False)
    cs25 = nc.declare_dram_parameter("cs25", [1, N], F32, isOutput=False)

    avt_cls_o = nc.declare_dram_parameter("avt_cls", [HD, N], F32, isOutput=True)
    avt_reg_o = nc.declare_dram_parameter("avt_reg", [HD, N], F32, isOutput=True)
    vt_cls_o = nc.declare_dram_parameter("vt_cls", [HD, N], F32, isOutput=True)
    vt_reg_o = nc.declare_dram_parameter("vt_reg", [HD, N], F32, isOutput=True)
    sim_o = nc.declare_dram_parameter("sim_rows", [N // NCORES, N], F32, isOutput=True)
    obj_o = nc.declare_dram_parameter("obj_rows", [N // NCORES, N], F32, isOutput=True)

    rg = [list(range(NCORES))]
    AF = mybir.ActivationFunctionType
    OP = mybir.AluOpType
    X = mybir.AxisListType.X

    with tile.TileContext(nc) as tc:
        with (
            tc.tile_pool(name="pc", bufs=1) as pc,
            tc.tile_pool(name="dram", bufs=1, space="DRAM") as dram,
        ):
            # ---------- static tiles ----------
            ident_bf = pc.tile([128, 128], BF16, tag="ident_bf")
            make_identity(nc, ident_bf)
            ident_f32 = pc.tile([128, 128], F32, tag="ident_f32")
            make_identity(nc, ident_f32)
            ones_f = pc.tile([128, 1], F32, tag="ones_f")
            nc.vector.memset(ones_f, 1.0)
            ones_col = pc.tile([128, 1], F32R, tag="ones")
            nc.vector.tensor_copy(ones_col[:], ones_f[:])
            cs_t = pc.tile([1, N], F32, tag="cs")
            nc.sync.dma_start(out=cs_t, in_=cs25[:, :])

            # long-lived per-head tensors
            qt = {}    # f32r  q^T (unnormalized)
            kp = {}    # f32r  scaled k'^T
            vn = {}    # bf16  normalized v^T
            vnat = {}  # bf16  V natural, block j at [:, j*128:(j+1)*128]
            rq = {}    # [128, 16] f32 per-partition 1/|q|
            for nm in ("cls", "reg"):
                qt[nm] = pc.tile([128, N], F32R, tag=f"qt_{nm}", name=f"qt_{nm}")
                kp[nm] = pc.tile([128, N], F32R, tag=f"kp_{nm}", name=f"kp_{nm}")
                vn[nm] = pc.tile([128, N], BF16, tag=f"vn_{nm}", name=f"vn_{nm}")
                vnat[nm] = pc.tile([128, N], BF16, tag=f"vnat_{nm}", name=f"vnat_{nm}")
                rq[nm] = pc.tile([128, 16], F32, tag=f"rq_{nm}", name=f"rq_{nm}")

            rq_scratch = dram.tile([2, N], F32, name="rq_scratch")

            # ================= phase A+B: projections & norms =============
            with (
                tc.tile_pool(name="early", bufs=1) as early,
                tc.tile_pool(name="xt", bufs=3) as xtp,
                tc.tile_pool(name="sqp", bufs=2) as sqp,
                tc.tile_pool(name="ps_a", bufs=1, space="PSUM") as ps_a,
            ):
                for ii, nm in enumerate(("cls", "reg")):
                    xpar = xt_cls if nm == "cls" else xt_reg
                    wpar = wt_cls if nm == "cls" else wt_reg
                    wts = early.tile([128, 8, 3 * HD], BF16, tag=f"wt_{nm}",
                                     name=f"wt_{nm}")
                    nc.sync.dma_start(
                        out=wts, in_=wpar.ap().rearrange("(c8 p) d -> p c8 d",
                                                         p=128))
                    vt = early.tile([128, N], F32, tag=f"vt_{nm}", name=f"vt_{nm}")
                    ssq = {}
                    for t in ("q", "k", "v"):
                        ssq[t] = early.tile([1, N], F32, tag=f"ssq_{nm}_{t}",
                                            name=f"ssq_{nm}_{t}")
                    dst = {"q": qt[nm], "k": kp[nm], "v": vt}
                    for half in range(2):
                        pps = {}
                        for pi in range(3):
                            for nl in range(2):
                                pps[pi, nl] = ps_a.tile([128, 512], F32, tag=f"proj{pi}{nl}", name=f"pps{pi}{nl}")
                        for c8 in range(8):
                            xc = xtp.tile([128, 1024], BF16, tag="xchunk")
                            nc.sync.dma_start(
                                out=xc,
                                in_=xpar[c8 * 128:(c8 + 1) * 128,
                                         half * 1024:(half + 1) * 1024])
                            for nl in range(2):
                                for pi in range(3):
                                    nc.tensor.matmul(
                                        pps[pi, nl][:],
                                        wts[:, c8, pi * HD:(pi + 1) * HD],
                                        xc[:, nl * 512:(nl + 1) * 512],
                                        start=(c8 == 0), stop=(c8 == 7))
                        for pi, t in enumerate(("q", "k", "v")):
                            for nl in range(2):
                                n4 = half * 2 + nl
                                sl = slice(n4 * 512, (n4 + 1) * 512)
                                sq = sqp.tile([128, 512], F32R, tag=f"sq_{nm}")
                                nc.scalar.activation(out=sq[:], in_=pps[pi, nl][:],
                                                     func=AF.Square)
                                nc.any.tensor_copy(out=dst[t][:, sl],
                                                   in_=pps[pi, nl][:])
                                sps = ps_a.tile([1, 512], F32, tag="misc_ps", bufs=2)
                                nc.tensor.matmul(sps[:], ones_col[:], sq[:],
                                                 start=True, stop=True)
                                nc.any.tensor_copy(out=ssq[t][:, sl],
                                                   in_=sps[:])

                    # ---------- norms for this input ----------
                    # k' = k * (25*cs/|k|) [cls]  or  k * (25/|k|) [reg]
                    rk = early.tile([1, N], F32, tag=f"rowa_{nm}", name="rk")
                    nc.scalar.activation(out=rk[:], in_=ssq["k"][:], func=AF.Sqrt)
                    nc.vector.reciprocal(rk[:], rk[:])
                    bk = early.tile([1, N], F32, tag=f"rowb_{nm}", name="bk")
                    if nm == "cls":
                        nc.vector.tensor_tensor(out=bk[:], in0=rk[:], in1=cs_t[:],
                                                op=OP.mult)
                    else:
                        nc.vector.tensor_scalar_mul(out=bk[:], in0=rk[:],
                                                    scalar1=SCALE)
                    bk_bc = early.tile([128, N], F32, tag="bc_big", name="bk_bc")
                    last_bcast = nc.gpsimd.partition_broadcast(bk_bc[:], bk[:])
                    nc.vector.tensor_tensor(out=kp[nm][:],
                                            in0=kp[nm][:].bitcast(F32),
                                            in1=bk_bc[:], op=OP.mult)

                    # vn = v / |v| (bf16)
                    rv = early.tile([1, N], F32, tag=f"rowc_{nm}", name="rv")
                    nc.scalar.activation(out=rv[:], in_=ssq["v"][:], func=AF.Sqrt)
                    nc.vector.reciprocal(rv[:], rv[:])
                    rv_bc = early.tile([128, N], F32, tag="bc_big", name="rv_bc")
                    last_bcast = nc.gpsimd.partition_broadcast(rv_bc[:], rv[:])
                    nc.vector.tensor_tensor(out=vn[nm][:], in0=vt[:],
                                            in1=rv_bc[:], op=OP.mult)

                    # rq: 1/|q_n| rearranged to partition-major [128, 16]
                    nc.sync.dma_start(out=rq_scratch[ii], in_=ssq["q"][0:1, :])
                    nc.sync.dma_start(
                        out=rq[nm],
                        in_=rq_scratch[ii].rearrange("(j p) -> p j", p=128))
                    nc.scalar.activation(out=rq[nm][:], in_=rq[nm][:], func=AF.Sqrt)
                    nc.vector.reciprocal(rq[nm][:], rq[nm][:])

                    # V natural (bf16): PE transpose of raw v^T
                    for j in range(NCH):
                        tps = ps_a.tile([128, 128], F32, tag="misc_ps", bufs=2)
                        nc.tensor.transpose(
                            tps[:], vt[:, j * 128:(j + 1) * 128], ident_f32[:])
                        nc.any.tensor_copy(
                            out=vnat[nm][:, j * 128:(j + 1) * 128], in_=tps[:])

                    # raw v^T is a kernel output
                    nc.sync.dma_start(
                        out=(vt_cls_o if nm == "cls" else vt_reg_o)[:, :],
                        in_=vt[:])

            # ---------- collectives: AllGather + AllToAll of vn ----------
            # Keep the Pool queue clear for phase-B broadcasts: every
            # collective waits on the last partition_broadcast so the Tile
            # scheduler cannot hoist a long AllGather ahead of them.
            from concourse.bass import _add_dep_helper
            NMI = {"cls": 0, "reg": 1}
            # one merged AllGather: rank block = [nm, 128, N]
            agi = dram.tile([2, 128, N], BF16, name="agi")
            for nm in ("cls", "reg"):
                nc.sync.dma_start(out=agi[NMI[nm]], in_=vn[nm][:])
            ag_out = dram.tile([NCORES, 2, 128, N], BF16, name="ago",
                               addr_space="Shared")
            cc = nc.gpsimd.collective_compute(
                "AllGather", OP.bypass, replica_groups=rg,
                ins=[agi[:].opt()], outs=[ag_out[:].opt()])
            _add_dep_helper(cc.ins, last_bcast.ins, sync=True,
                            reason="collectives after phase-B broadcasts")

            # one merged AllToAll: dest block j = both nm's columns
            # {g*512 + j*64 .. +64 : g in 0..3}
            a2i = dram.tile([NCORES, 2, 128, NGRP, RPG], BF16, name="a2i")
            for nm in ("cls", "reg"):
                for j in range(NCORES):
                    for g in range(NGRP):
                        c0 = g * GROWS + j * RPG
                        nc.sync.dma_start(out=a2i[j, NMI[nm], :, g, :],
                                          in_=vn[nm][:, c0:c0 + RPG])
            a2a_out = dram.tile([NCORES, 2, 128, NGRP, RPG], BF16, name="a2o")
            cc = nc.gpsimd.collective_compute(
                "AllToAll", OP.bypass, replica_groups=rg,
                ins=[a2i[:].opt()], outs=[a2a_out[:].opt()])
            _add_dep_helper(cc.ins, last_bcast.ins, sync=True,
                            reason="collectives after phase-B broadcasts")

            # ================= phase C: attention =========================
            att_bounce = [dram.tile([GROWS, N], BF16, name=f"attb_{g}") for g in range(NGRP)]
            rs_out = [dram.tile([RPG, N], BF16, name=f"rsout_{g}") for g in range(NGRP)]

            with (
                tc.tile_pool(name="cw", bufs=1) as cw,
                tc.tile_pool(name="attn", bufs=9) as attnp,
                tc.tile_pool(name="attnT", bufs=2) as attnTp,
                tc.tile_pool(name="avsb", bufs=2) as avsb,
                tc.tile_pool(name="ps_c", bufs=3, space="PSUM") as ps_c,
                tc.tile_pool(name="ps_t", bufs=1, space="PSUM") as ps_t,
                tc.tile_pool(name="ps_av", bufs=1, space="PSUM") as ps_av,
            ):
                attn_chunks = []
                atw = None
                for i in range(NCH):
                    g = i // 4
                    if i % 4 == 0:
                        atw = attnTp.tile([128, NCH * 512], BF16, tag="attnT",
                                          name=f"atw_{g}")
                    ec = {}
                    rec = {}
                    for nm in ("cls", "reg"):
                        e = cw.tile([128, N], BF16, tag=f"e_{nm}", name=f"e_{nm}", bufs=2)
                        parts = cw.tile([128, 2], F32, tag=f"parts_{nm}", name=f"parts_{nm}", bufs=3)
                        for mh in range(2):
                            # two-bank score psum: 2 matmuls, ONE exp over
                            # [128, 1024] (halves the per-chunk hop count)
                            sps = ps_c.tile([128, 1024], F32, tag="score")
                            for mq in range(2):
                                m4 = mh * 2 + mq
                                nc.tensor.matmul(
                                    sps[:, mq * 512:(mq + 1) * 512],
                                    qt[nm][:, i * 128:(i + 1) * 128],
                                    kp[nm][:, m4 * 512:(m4 + 1) * 512],
                                    start=True, stop=True)
                            nc.scalar.activation(
                                out=e[:, mh * 1024:(mh + 1) * 1024], in_=sps[:],
                                func=AF.Exp, scale=rq[nm][:, i:i + 1],
                                accum_out=parts[:, mh:mh + 1])
                        rs_sum = cw.tile([128, 1], F32, tag=f"rssum_{nm}", name=f"rssum_{nm}", bufs=3)
                        nc.vector.reduce_sum(out=rs_sum[:], in_=parts[:], axis=X)
                        rc = cw.tile([128, 1], F32, tag=f"rec_{nm}", name=f"rec_{nm}", bufs=3)
                        nc.vector.reciprocal(rc[:], rs_sum[:])
                        ec[nm] = e
                        rec[nm] = rc

                    t1 = cw.tile([128, N], BF16, tag="t1")
                    nc.vector.tensor_scalar(
                        out=t1[:], in0=ec["cls"][:], scalar1=rec["cls"][:],
                        scalar2=0.5, op0=OP.mult, op1=OP.mult)
                    t2 = cw.tile([128, N], BF16, tag="t2")
                    nc.vector.tensor_scalar(
                        out=t2[:], in0=ec["reg"][:], scalar1=rec["reg"][:],
                        scalar2=0.5, op0=OP.mult, op1=OP.mult)
                    ab = attnp.tile([128, N], BF16, tag="attn_bf")
                    nc.vector.tensor_tensor(out=ab[:], in0=t1[:], in1=t2[:],
                                            op=OP.add)
                    attn_chunks.append(ab)
                    nc.sync.dma_start(
                        out=att_bounce[g][(i % 4) * 128:(i % 4 + 1) * 128, :],
                        in_=ab[:])

                    if i % 4 == 3:
                        # ReduceScatter for this group of 512 rows
                        nc.gpsimd.collective_compute(
                            "ReduceScatter", OP.add, replica_groups=rg,
                            ins=[att_bounce[g][:].opt()],
                            outs=[rs_out[g][:].opt()])

                        # transpose the 4 chunks -> attnT window [m, 512]
                        for j in range(NCH):
                            tps = ps_t.tile([128, 512], BF16, tag="attr")
                            for ii in range(4):
                                nc.tensor.transpose(
                                    tps[:, ii * 128:(ii + 1) * 128],
                                    attn_chunks[g * 4 + ii][:, j * 128:(j + 1) * 128],
                                    ident_bf[:])
                            nc.any.tensor_copy(
                                out=atw[:, j * 512:(j + 1) * 512], in_=tps[:])

                        # AV for these 512 output columns
                        for nm, opar in (("cls", avt_cls_o), ("reg", avt_reg_o)):
                            aps = ps_av.tile([128, 512], F32, tag="av")
                            for j in range(NCH):
                                nc.tensor.matmul(
                                    aps[:],
                                    vnat[nm][:, j * 128:(j + 1) * 128],
                                    atw[:, j * 512:(j + 1) * 512],
                                    start=(j == 0), stop=(j == NCH - 1))
                            av_s = avsb.tile([128, 512], F32, tag="av_sb")
                            nc.any.tensor_copy(out=av_s[:], in_=aps[:])
                            nc.sync.dma_start(
                                out=opar[:, g * 512:(g + 1) * 512], in_=av_s[:])

            # ================= phase D: gram row-slices + masks ===========
            with (
                tc.tile_pool(name="late", bufs=1) as late,
                tc.tile_pool(name="vgp", bufs=3) as vgp,
                tc.tile_pool(name="pe_big", bufs=3) as peb,
                tc.tile_pool(name="ps_g", bufs=1, space="PSUM") as ps_g,
            ):
                masks = {}
                for nm, thr in (("cls", SIM_THRESH * H), ("reg", CONF_SIM_THRESH * H)):
                    a2s = late.tile([128, NCORES * NGRP * RPG], BF16, tag=f"a2s_{nm}", name=f"a2s_{nm}")
                    nc.sync.dma_start(
                        out=a2s[:].rearrange("p (h g r) -> p h g r",
                                             h=NCORES, g=NGRP),
                        in_=a2a_out[:, NMI[nm]].rearrange("h p g r -> p h g r"))
                    mask = late.tile([128, 2, N], F32, tag=f"mask_{nm}", name=f"mask_{nm}")
                    gt = {}
                    for mm in range(2):
                        for m4 in range(4):
                            gt[mm, m4] = ps_g.tile([128, 512], F32, tag=f"gram{mm}{m4}", name=f"gram{mm}{m4}")
                    for c8 in range(8):
                        vg = vgp.tile([128, N], BF16, tag="vg")
                        nc.sync.dma_start(out=vg, in_=ag_out[c8, NMI[nm]])
                        for mm in range(2):
                            for m4 in range(4):
                                nc.tensor.matmul(
                                    gt[mm, m4][:],
                                    a2s[:, c8 * 256 + mm * 128:
                                        c8 * 256 + (mm + 1) * 128],
                                    vg[:, m4 * 512:(m4 + 1) * 512],
                                    start=(c8 == 0), stop=(c8 == 7))
                    for mm in range(2):
                        for m4 in range(4):
                            nc.vector.tensor_scalar(
                                out=mask[:, mm, m4 * 512:(m4 + 1) * 512],
                                in0=gt[mm, m4][:], scalar1=float(thr),
                                scalar2=None, op0=OP.is_gt)
                    masks[nm] = mask

                # ---------- phase E: sim_round2 / obj_mask rows ----------
                for mm in range(2):
                    srows = late.tile([128, N], BF16, tag="srows")
                    for gg in range(2):
                        g = mm * 2 + gg
                        nc.sync.dma_start(out=srows[gg * RPG:(gg + 1) * RPG, :],
                                          in_=rs_out[g][:, :])
                    es = peb.tile([128, N], F32, tag="pe_big")
                    essum = late.tile([128, 1], F32, tag="essum")
                    nc.scalar.activation(out=es[:], in_=srows[:], func=AF.Exp,
                                         scale=1.0 / H, accum_out=essum[:])
                    # sim2 = es * mask_cls / rowsum(es * mask_cls); the
                    # pre-mask softmax denominator cancels in the renorm.
                    msk = peb.tile([128, N], F32, tag="pe_big")
                    nc.vector.tensor_tensor(out=msk[:], in0=es[:],
                                            in1=masks["cls"][:, mm, :], op=OP.mult)
                    mrow = late.tile([128, 1], F32, tag="mrow")
                    nc.vector.reduce_sum(out=mrow[:], in_=msk[:], axis=X)
                    rmrow = late.tile([128, 1], F32, tag="rmrow")
                    nc.vector.reciprocal(rmrow[:], mrow[:])
                    sim2 = peb.tile([128, N], F32, tag="pe_big")
                    nc.vector.tensor_scalar(out=sim2[:], in0=msk[:],
                                            scalar1=rmrow[:], scalar2=None,
                                            op0=OP.mult)
                    nc.sync.dma_start(out=sim_o[mm * 128:(mm + 1) * 128, :],
                                      in_=sim2[:])

                    omsk = peb.tile([128, N], F32, tag="pe_big")
                    nc.vector.tensor_tensor(out=omsk[:], in0=sim2[:],
                                            in1=masks["reg"][:, mm, :], op=OP.mult)
                    orow = late.tile([128, 1], F32, tag="orow")
                    nc.vector.reduce_sum(out=orow[:], in_=omsk[:], axis=X)
                    rorow = late.tile([128, 1], F32, tag="rorow")
                    nc.vector.reciprocal(rorow[:], orow[:])
                    obj2 = peb.tile([128, N], F32, tag="pe_big")
                    nc.vector.tensor_scalar(out=obj2[:], in0=omsk[:],
                                            scalar1=rorow[:], scalar2=None,
                                            op0=OP.mult)
                    nc.sync.dma_start(out=obj_o[mm * 128:(mm + 1) * 128, :],
                                      in_=obj2[:])

    nc.compile()
    return nc


_NC_CACHE = None


def _get_program():
    global _NC_CACHE
    if _NC_CACHE is None:
        _NC_CACHE = build_program()
    return _NC_CACHE


def make_in_maps(x_cls, x_reg, cls_score, W_qkv_cls, W_qkv_reg):
    import ml_dtypes
    bf = ml_dtypes.bfloat16
    xt_cls = np.ascontiguousarray(x_cls[0].T).astype(bf)   # [C, N]
    xt_reg = np.ascontiguousarray(x_reg[0].T).astype(bf)
    cs25 = (SCALE * cls_score).reshape(1, N).astype(np.float32)
    in_maps = []
    for h in range(NCORES):
        rows = np.r_[h * HD:(h + 1) * HD,
                     C + h * HD:C + (h + 1) * HD,
                     2 * C + h * HD:2 * C + (h + 1) * HD]
        in_maps.append({
            "xt_cls": xt_cls,
            "xt_reg": xt_reg,
            "wt_cls": np.ascontiguousarray(W_qkv_cls[rows].T).astype(bf),
            "wt_reg": np.ascontiguousarray(W_qkv_reg[rows].T).astype(bf),
            "cs25": cs25,
        })
    return in_maps


def assemble(results):
    """results: list over cores of dicts of np arrays -> full outputs."""
    out_cls = np.empty((1, N, 2 * C), np.float32)
    out_reg = np.empty((1, N, 2 * C), np.float32)
    sim = np.empty((N, N), np.float32)
    obj = np.empty((N, N), np.float32)
    for h in range(NCORES):
        r = results[h]
        out_cls[0, :, h * HD:(h + 1) * HD] = r["avt_cls"].T
        out_cls[0, :, C + h * HD:C + (h + 1) * HD] = r["vt_cls"].T
        out_reg[0, :, h * HD:(h + 1) * HD] = r["avt_reg"].T
        out_reg[0, :, C + h * HD:C + (h + 1) * HD] = r["vt_reg"].T
        # device row order for rank h: for g in 0..3: rows g*512+h*64 .. +64
        ridx = np.concatenate([
            np.arange(g * GROWS + h * RPG, g * GROWS + (h + 1) * RPG)
            for g in range(NGRP)])
        sim[ridx] = r["sim_rows"]
        obj[ridx] = r["obj_rows"]
    return out_cls, out_reg, sim, obj


def kernel(x_cls, x_reg, cls_score, fg_score, W_qkv_cls, W_qkv_reg):
    x_cls = np.asarray(x_cls, np.float32)
    x_reg = np.asarray(x_reg, np.float32)
    cls_score = np.asarray(cls_score, np.float32)
    W_qkv_cls = np.asarray(W_qkv_cls, np.float32)
    W_qkv_reg = np.asarray(W_qkv_reg, np.float32)

    nc = _get_program()
    in_maps = make_in_maps(x_cls, x_reg, cls_score, W_qkv_cls, W_qkv_reg)
    res = run_bass_kernel_spmd(nc, in_maps, core_ids=list(range(NCORES)))
    return assemble(res.results)


# revision 33
# speedup vs baseline: 1.0724x; 1.0010x over previous
"""Trainium2 Bass kernel for nn_Attention_aware_msa (sparse_attention).

Sharding: one attention head per NeuronCore (8 heads / 8 cores), per the
tensor-parallel hint.  All cross-core traffic is done with collectives:

  - 4 chunked ReduceScatters produce each rank's row-slice of the
    head-summed attention (for sim_round2).
  - An AllGather of the L2-normalized v^T (bf16) gives every core the full
    stacked Vn^T [1024, 2048]; Sum_h vn_h vn_h^T == Vn Vn^T, so each core
    computes only its row-slice of the head-summed gram matrices (sim/obj
    masks) locally.
  - An AllToAll delivers Vn^T[:, my_rows] as the stationary operand for
    that row-slice without any rank-dependent addressing.

Compute layout: QKV projections contract C=1024 on the PE with bf16
operands (x and W are shipped as bf16; fp32 PSUM accumulation), producing
q/k/v directly in transposed [d=128, n] layout; scores and attn@v run with
f32r / bf16 operands at full PE rate.  Scores fold all scaling into the operands:
k' columns are pre-scaled by 25*cs[m]/|k_m| (cls) / 25/|k_m| (reg), and
the 1/|q_n| row factor rides the ACT exp's per-partition scale AP.  exp is
computed without row-max subtraction (|logits| <= 25, safe in f32).
attn = 0.5*(exp_c/rowsum_c + exp_r/rowsum_r) in bf16; attn @ v is computed
transposed (out[d, n]) with V-natural stationary tiles and PE-transposed
attn as the moving operand.

Host marshals x^T and per-head weight slices in, and reassembles the four
outputs (out_cls, out_reg, sim_round2, obj_mask) from per-core pieces.
"""
import numpy as np

try:
    import jax
    jax.config.update("jax_compilation_cache_dir", "/tmp/bass_jax_cache")
    jax.config.update("jax_persistent_cache_min_compile_time_secs", 0.0)
except Exception:
    pass

import concourse.bacc as bacc
import concourse.tile as tile
from concourse import mybir
from concourse.bass_utils import run_bass_kernel_spmd
from concourse.masks import make_identity

F32 = mybir.dt.float32
F32R = mybir.dt.float32r
BF16 = mybir.dt.bfloat16

B, N, C, H = 1, 2048, 1024, 8
HD = C // H            # 128 head dim
NCORES = 8
SCALE = 25.0
SIM_THRESH = 0.75
CONF_SIM_THRESH = 0.99
NCH = N // 128         # 16 row chunks of 128
NGRP = 4               # chunk groups (512 rows each) -> 4 ReduceScatters
GROWS = N // NGRP      # 512 rows per group
RPG = GROWS // NCORES  # 64 rows per rank per group


def build_program():
    nc = bacc.Bacc("TRN2", target_bir_lowering=False, debug=False,
                   num_devices=NCORES)

    # ---- parameters ----
    xt_cls = nc.declare_dram_parameter("xt_cls", [C, N], BF16, isOutput=False)
    xt_reg = nc.declare_dram_parameter("xt_reg", [C, N], BF16, isOutput=False)
    wt_cls = nc.declare_dram_parameter("wt_cls", [C, 3 * HD], BF16, isOutput=False)
    wt_reg = nc.declare_dram_parameter("wt_reg", [C, 3 * HD], BF16, isOutput=False)
    cs25 = nc.declare_dram_parameter("cs25", [1, N], F32, isOutput=False)

    avt_cls_o = nc.declare_dram_parameter("avt_cls", [HD, N], F32, isOutput=True)
    avt_reg_o = nc.declare_dram_parameter("avt_reg", [HD, N], F32, isOutput=True)
    vt_cls_o = nc.declare_dram_parameter("vt_cls", [HD, N], F32, isOutput=True)
    vt_reg_o = nc.declare_dram_parameter("vt_reg", [HD, N], F32, isOutput=True)
    sim_o = nc.declare_dram_parameter("sim_rows", [N // NCORES, N], F32, isOutput=True)
    obj_o = nc.declare_dram_parameter("obj_rows", [N // NCORES, N], F32, isOutput=True)

    rg = [list(range(NCORES))]
    AF = mybir.ActivationFunctionType
    OP = mybir.AluOpType
    X = mybir.AxisListType.X

    with tile.TileContext(nc) as tc:
        with (
            tc.tile_pool(name="pc", bufs=1) as pc,
            tc.tile_pool(name="dram", bufs=1, space="DRAM") as dram,
        ):
            # ---------- static tiles ----------
            ident_bf = pc.tile([128, 128], BF16, tag="ident_bf")
            make_identity(nc, ident_bf)
            ident_f32 = pc.tile([128, 128], F32, tag="ident_f32")
            make_identity(nc, ident_f32)
            ones_f = pc.tile([128, 1], F32, tag="ones_f")
            nc.vector.memset(ones_f, 1.0)
            ones_col = pc.tile([128, 1], F32R, tag="ones")
            nc.vector.tensor_copy(ones_col[:], ones_f[:])
            cs_t = pc.tile([1, N], F32, tag="cs")
            nc.sync.dma_start(out=cs_t, in_=cs25[:, :])

            # long-lived per-head tensors
            qt = {}    # f32r  q^T (unnormalized)
            kp = {}    # f32r  scaled k'^T
            vn = {}    # bf16  normalized v^T
            vnat = {}  # bf16  V natural, block j at [:, j*128:(j+1)*128]
            rq = {}    # [128, 16] f32 per-partition 1/|q|
            for nm in ("cls", "reg"):
                qt[nm] = pc.tile([128, N], F32R, tag=f"qt_{nm}", name=f"qt_{nm}")
                kp[nm] = pc.tile([128, N], F32R, tag=f"kp_{nm}", name=f"kp_{nm}")
                vn[nm] = pc.tile([128, N], BF16, tag=f"vn_{nm}", name=f"vn_{nm}")
                vnat[nm] = pc.tile([128, N], BF16, tag=f"vnat_{nm}", name=f"vnat_{nm}")
                rq[nm] = pc.tile([128, 16], F32, tag=f"rq_{nm}", name=f"rq_{nm}")

            rq_scratch = dram.tile([2, N], F32, name="rq_scratch")

            # ================= phase A+B: projections & norms =============
            with (
                tc.tile_pool(name="early", bufs=1) as early,
                tc.tile_pool(name="xt", bufs=3) as xtp,
                tc.tile_pool(name="sqp", bufs=2) as sqp,
                tc.tile_pool(name="ps_a", bufs=1, space="PSUM") as ps_a,
            ):
                for ii, nm in enumerate(("cls", "reg")):
                    xpar = xt_cls if nm == "cls" else xt_reg
                    wpar = wt_cls if nm == "cls" else wt_reg
                    wts = early.tile([128, 8, 3 * HD], BF16, tag=f"wt_{nm}",
                                     name=f"wt_{nm}")
                    nc.sync.dma_start(
                        out=wts, in_=wpar.ap().rearrange("(c8 p) d -> p c8 d",
                                                         p=128))
                    vt = early.tile([128, N], F32, tag=f"vt_{nm}", name=f"vt_{nm}")
                    ssq = {}
                    for t in ("q", "k", "v"):
                        ssq[t] = early.tile([1, N], F32, tag=f"ssq_{nm}_{t}",
                                            name=f"ssq_{nm}_{t}")
                    dst = {"q": qt[nm], "k": kp[nm], "v": vt}
                    for half in range(2):
                        pps = {}
                        for pi in range(3):
                            for nl in range(2):
                                pps[pi, nl] = ps_a.tile([128, 512], F32, tag=f"proj{pi}{nl}", name=f"pps{pi}{nl}")
                        for c8 in range(8):
                            xc = xtp.tile([128, 1024], BF16, tag="xchunk")
                            nc.sync.dma_start(
                                out=xc,
                                in_=xpar[c8 * 128:(c8 + 1) * 128,
                                         half * 1024:(half + 1) * 1024])
                            for nl in range(2):
                                for pi in range(3):
                                    nc.tensor.matmul(
                                        pps[pi, nl][:],
                                        wts[:, c8, pi * HD:(pi + 1) * HD],
                                        xc[:, nl * 512:(nl + 1) * 512],
                                        start=(c8 == 0), stop=(c8 == 7))
                        for pi, t in enumerate(("k", "q", "v")):
                            for nl in range(2):
                                n4 = half * 2 + nl
                                sl = slice(n4 * 512, (n4 + 1) * 512)
                                ppt = pps[("q", "k", "v").index(t), nl]
                                sq = sqp.tile([128, 512], F32R, tag=f"sq_{nm}")
                                nc.scalar.activation(out=sq[:], in_=ppt[:],
                                                     func=AF.Square)
                                nc.any.tensor_copy(out=dst[t][:, sl],
                                                   in_=ppt[:])
                                sps = ps_a.tile([1, 512], F32, tag="misc_ps", bufs=2)
                                nc.tensor.matmul(sps[:], ones_col[:], sq[:],
                                                 start=True, stop=True)
                                nc.any.tensor_copy(out=ssq[t][:, sl],
                                                   in_=sps[:])

                                if t == "k":
                                    # pipeline k' = k * (25*[cs]/|k|) per
                                    # 512-column slice as soon as its column
                                    # sums land -- kp readiness gates the
                                    # whole attention phase.
                                    bk4 = early.tile([1, 512], F32,
                                                     tag=f"bk4_{nm}",
                                                     name="bk4", bufs=2)
                                    nc.scalar.activation(out=bk4[:],
                                                         in_=ssq["k"][0:1, sl],
                                                         func=AF.Sqrt)
                                    nc.vector.reciprocal(bk4[:], bk4[:])
                                    if nm == "cls":
                                        nc.vector.tensor_tensor(
                                            out=bk4[:], in0=bk4[:],
                                            in1=cs_t[0:1, sl], op=OP.mult)
                                    else:
                                        nc.vector.tensor_scalar_mul(
                                            out=bk4[:], in0=bk4[:],
                                            scalar1=SCALE)
                                    bk_bc = early.tile([128, 512], F32,
                                                       tag="bc_big", name="bk_bc",
                                                       bufs=2)
                                    last_bcast = nc.gpsimd.partition_broadcast(
                                        bk_bc[:], bk4[:])
                                    nc.vector.tensor_tensor(
                                        out=kp[nm][:, sl],
                                        in0=kp[nm][:, sl].bitcast(F32),
                                        in1=bk_bc[:], op=OP.mult)

                    # ---------- norms for this input ----------

                    # vn = v / |v| (bf16)
                    rv = early.tile([1, N], F32, tag=f"rowc_{nm}", name="rv")
                    nc.scalar.activation(out=rv[:], in_=ssq["v"][:], func=AF.Sqrt)
                    nc.vector.reciprocal(rv[:], rv[:])
                    rv_bc = early.tile([128, N], F32, tag="bc_big", name="rv_bc", bufs=2)
                    last_bcast = nc.gpsimd.partition_broadcast(rv_bc[:], rv[:])
                    nc.vector.tensor_tensor(out=vn[nm][:], in0=vt[:],
                                            in1=rv_bc[:], op=OP.mult)

                    # rq: 1/|q_n| rearranged to partition-major [128, 16]
                    nc.sync.dma_start(out=rq_scratch[ii], in_=ssq["q"][0:1, :])
                    nc.sync.dma_start(
                        out=rq[nm],
                        in_=rq_scratch[ii].rearrange("(j p) -> p j", p=128))
                    nc.scalar.activation(out=rq[nm][:], in_=rq[nm][:], func=AF.Sqrt)
                    nc.vector.reciprocal(rq[nm][:], rq[nm][:])

                    # V natural (bf16): PE transpose of raw v^T
                    for j in range(NCH):
                        tps = ps_a.tile([128, 128], F32, tag="misc_ps", bufs=2)
                        nc.tensor.transpose(
                            tps[:], vt[:, j * 128:(j + 1) * 128], ident_f32[:])
                        nc.any.tensor_copy(
                            out=vnat[nm][:, j * 128:(j + 1) * 128], in_=tps[:])

                    # raw v^T is a kernel output
                    nc.sync.dma_start(
                        out=(vt_cls_o if nm == "cls" else vt_reg_o)[:, :],
                        in_=vt[:])

            # ---------- collectives: AllGather + AllToAll of vn ----------
            # Keep the Pool queue clear for phase-B broadcasts: every
            # collective waits on the last partition_broadcast so the Tile
            # scheduler cannot hoist a long AllGather ahead of them.
            from concourse.bass import _add_dep_helper
            NMI = {"cls": 0, "reg": 1}
            # one merged AllGather: rank block = [nm, 128, N]
            agi = dram.tile([2, 128, N], BF16, name="agi")
            for nm in ("cls", "reg"):
                nc.sync.dma_start(out=agi[NMI[nm]], in_=vn[nm][:])
            ag_out = dram.tile([NCORES, 2, 128, N], BF16, name="ago",
                               addr_space="Shared")
            cc = nc.gpsimd.collective_compute(
                "AllGather", OP.bypass, replica_groups=rg,
                ins=[agi[:].opt()], outs=[ag_out[:].opt()])
            _add_dep_helper(cc.ins, last_bcast.ins, sync=True,
                            reason="collectives after phase-B broadcasts")

            # one merged AllToAll: dest block j = both nm's columns
            # {g*512 + j*64 .. +64 : g in 0..3}
            a2i = dram.tile([NCORES, 2, 128, NGRP, RPG], BF16, name="a2i")
            for nm in ("cls", "reg"):
                for j in range(NCORES):
                    for g in range(NGRP):
                        c0 = g * GROWS + j * RPG
                        nc.sync.dma_start(out=a2i[j, NMI[nm], :, g, :],
                                          in_=vn[nm][:, c0:c0 + RPG])
            a2a_out = dram.tile([NCORES, 2, 128, NGRP, RPG], BF16, name="a2o")
            cc = nc.gpsimd.collective_compute(
                "AllToAll", OP.bypass, replica_groups=rg,
                ins=[a2i[:].opt()], outs=[a2a_out[:].opt()])
            _add_dep_helper(cc.ins, last_bcast.ins, sync=True,
                            reason="collectives after phase-B broadcasts")

            # ================= phase C: attention =========================
            att_bounce = [dram.tile([GROWS, N], BF16, name=f"attb_{g}") for g in range(NGRP)]
            rs_out = [dram.tile([RPG, N], BF16, name=f"rsout_{g}") for g in range(NGRP)]

            with (
                tc.tile_pool(name="cw", bufs=1) as cw,
                tc.tile_pool(name="attn", bufs=9) as attnp,
                tc.tile_pool(name="attnT", bufs=2) as attnTp,
                tc.tile_pool(name="avsb", bufs=2) as avsb,
                tc.tile_pool(name="ps_c", bufs=3, space="PSUM") as ps_c,
                tc.tile_pool(name="ps_t", bufs=1, space="PSUM") as ps_t,
                tc.tile_pool(name="ps_av", bufs=1, space="PSUM") as ps_av,
            ):
                attn_chunks = []
                atw = None
                for i in range(NCH):
                    g = i // 4
                    if i % 4 == 0:
                        atw = attnTp.tile([128, NCH * 512], BF16, tag="attnT",
                                          name=f"atw_{g}")
                    ec = {}
                    rec = {}
                    for nm in ("cls", "reg"):
                        e = cw.tile([128, N], BF16, tag=f"e_{nm}", name=f"e_{nm}", bufs=2)
                        parts = cw.tile([128, 2], F32, tag=f"parts_{nm}", name=f"parts_{nm}", bufs=3)
                        for mh in range(2):
                            # two-bank score psum: 2 matmuls, ONE exp over
                            # [128, 1024] (halves the per-chunk hop count)
                            sps = ps_c.tile([128, 1024], F32, tag="score")
                            for mq in range(2):
                                m4 = mh * 2 + mq
                                nc.tensor.matmul(
                                    sps[:, mq * 512:(mq + 1) * 512],
                                    qt[nm][:, i * 128:(i + 1) * 128],
                                    kp[nm][:, m4 * 512:(m4 + 1) * 512],
                                    start=True, stop=True)
                            nc.scalar.activation(
                                out=e[:, mh * 1024:(mh + 1) * 1024], in_=sps[:],
                                func=AF.Exp, scale=rq[nm][:, i:i + 1],
                                accum_out=parts[:, mh:mh + 1])
                        rs_sum = cw.tile([128, 1], F32, tag=f"rssum_{nm}", name=f"rssum_{nm}", bufs=3)
                        nc.vector.reduce_sum(out=rs_sum[:], in_=parts[:], axis=X)
                        rc = cw.tile([128, 1], F32, tag=f"rec_{nm}", name=f"rec_{nm}", bufs=3)
                        nc.vector.reciprocal(rc[:], rs_sum[:])
                        ec[nm] = e
                        rec[nm] = rc

                    t1 = cw.tile([128, N], BF16, tag="t1")
                    nc.vector.tensor_scalar(
                        out=t1[:], in0=ec["cls"][:], scalar1=rec["cls"][:],
                        scalar2=0.5, op0=OP.mult, op1=OP.mult)
                    t2 = cw.tile([128, N], BF16, tag="t2")
                    nc.vector.tensor_scalar(
                        out=t2[:], in0=ec["reg"][:], scalar1=rec["reg"][:],
                        scalar2=0.5, op0=OP.mult, op1=OP.mult)
                    ab = attnp.tile([128, N], BF16, tag="attn_bf")
                    nc.vector.tensor_tensor(out=ab[:], in0=t1[:], in1=t2[:],
                                            op=OP.add)
                    attn_chunks.append(ab)
                    nc.sync.dma_start(
                        out=att_bounce[g][(i % 4) * 128:(i % 4 + 1) * 128, :],
                        in_=ab[:])

                    if i % 4 == 3:
                        # ReduceScatter for this group of 512 rows
                        nc.gpsimd.collective_compute(
                            "ReduceScatter", OP.add, replica_groups=rg,
                            ins=[att_bounce[g][:].opt()],
                            outs=[rs_out[g][:].opt()])

                        # transpose the 4 chunks -> attnT window [m, 512]
                        for j in range(NCH):
                            tps = ps_t.tile([128, 512], BF16, tag="attr")
                            for ii in range(4):
                                nc.tensor.transpose(
                                    tps[:, ii * 128:(ii + 1) * 128],
                                    attn_chunks[g * 4 + ii][:, j * 128:(j + 1) * 128],
                                    ident_bf[:])
                            nc.any.tensor_copy(
                                out=atw[:, j * 512:(j + 1) * 512], in_=tps[:])

                        # AV for these 512 output columns
                        for nm, opar in (("cls", avt_cls_o), ("reg", avt_reg_o)):
                            aps = ps_av.tile([128, 512], F32, tag="av")
                            for j in range(NCH):
                                nc.tensor.matmul(
                                    aps[:],
                                    vnat[nm][:, j * 128:(j + 1) * 128],
                                    atw[:, j * 512:(j + 1) * 512],
                                    start=(j == 0), stop=(j == NCH - 1))
                            av_s = avsb.tile([128, 512], F32, tag="av_sb")
                            nc.any.tensor_copy(out=av_s[:], in_=aps[:])
                            nc.sync.dma_start(
                                out=opar[:, g * 512:(g + 1) * 512], in_=av_s[:])

            # ================= phase D: gram row-slices + masks ===========
            with (
                tc.tile_pool(name="late", bufs=1) as late,
                tc.tile_pool(name="vgp", bufs=3) as vgp,
                tc.tile_pool(name="pe_big", bufs=3) as peb,
                tc.tile_pool(name="ps_g", bufs=1, space="PSUM") as ps_g,
            ):
                masks = {}
                for nm, thr in (("cls", SIM_THRESH * H), ("reg", CONF_SIM_THRESH * H)):
                    a2s = late.tile([128, NCORES * NGRP * RPG], BF16, tag=f"a2s_{nm}", name=f"a2s_{nm}")
                    nc.sync.dma_start(
                        out=a2s[:].rearrange("p (h g r) -> p h g r",
                                             h=NCORES, g=NGRP),
                        in_=a2a_out[:, NMI[nm]].rearrange("h p g r -> p h g r"))
                    mask = late.tile([128, 2, N], F32, tag=f"mask_{nm}", name=f"mask_{nm}")
                    gt = {}
                    for mm in range(2):
                        for m4 in range(4):
                            gt[mm, m4] = ps_g.tile([128, 512], F32, tag=f"gram{mm}{m4}", name=f"gram{mm}{m4}")
                    for c8 in range(8):
                        vg = vgp.tile([128, N], BF16, tag="vg")
                        nc.sync.dma_start(out=vg, in_=ag_out[c8, NMI[nm]])
                        for mm in range(2):
                            for m4 in range(4):
                                nc.tensor.matmul(
                                    gt[mm, m4][:],
                                    a2s[:, c8 * 256 + mm * 128:
                                        c8 * 256 + (mm + 1) * 128],
                                    vg[:, m4 * 512:(m4 + 1) * 512],
                                    start=(c8 == 0), stop=(c8 == 7))
                    for mm in range(2):
                        for m4 in range(4):
                            nc.vector.tensor_scalar(
                                out=mask[:, mm, m4 * 512:(m4 + 1) * 512],
                                in0=gt[mm, m4][:], scalar1=float(thr),
                                scalar2=None, op0=OP.is_gt)
                    masks[nm] = mask

                # ---------- phase E: sim_round2 / obj_mask rows ----------
                for mm in range(2):
                    srows = late.tile([128, N], BF16, tag="srows")
                    for gg in range(2):
                        g = mm * 2 + gg
                        nc.sync.dma_start(out=srows[gg * RPG:(gg + 1) * RPG, :],
                                          in_=rs_out[g][:, :])
                    es = peb.tile([128, N], F32, tag="pe_big")
                    essum = late.tile([128, 1], F32, tag="essum")
                    nc.scalar.activation(out=es[:], in_=srows[:], func=AF.Exp,
                                         scale=1.0 / H, accum_out=essum[:])
                    # sim2 = es * mask_cls / rowsum(es * mask_cls); the
                    # pre-mask softmax denominator cancels in the renorm.
                    msk = peb.tile([128, N], F32, tag="pe_big")
                    nc.vector.tensor_tensor(out=msk[:], in0=es[:],
                                            in1=masks["cls"][:, mm, :], op=OP.mult)
                    mrow = late.tile([128, 1], F32, tag="mrow")
                    nc.vector.reduce_sum(out=mrow[:], in_=msk[:], axis=X)
                    rmrow = late.tile([128, 1], F32, tag="rmrow")
                    nc.vector.reciprocal(rmrow[:], mrow[:])
                    sim2 = peb.tile([128, N], F32, tag="pe_big")
                    nc.vector.tensor_scalar(out=sim2[:], in0=msk[:],
                                            scalar1=rmrow[:], scalar2=None,
                                            op0=OP.mult)
                    nc.sync.dma_start(out=sim_o[mm * 128:(mm + 1) * 128, :],
                                      in_=sim2[:])

                    omsk = peb.tile([128, N], F32, tag="pe_big")
                    nc.vector.tensor_tensor(out=omsk[:], in0=sim2[:],
                                            in1=masks["reg"][:, mm, :], op=OP.mult)
                    orow = late.tile([128, 1], F32, tag="orow")
                    nc.vector.reduce_sum(out=orow[:], in_=omsk[:], axis=X)
                    rorow = late.tile([128, 1], F32, tag="rorow")
                    nc.vector.reciprocal(rorow[:], orow[:])
                    obj2 = peb.tile([128, N], F32, tag="pe_big")
                    nc.vector.tensor_scalar(out=obj2[:], in0=omsk[:],
                                            scalar1=rorow[:], scalar2=None,
                                            op0=OP.mult)
                    nc.sync.dma_start(out=obj_o[mm * 128:(mm + 1) * 128, :],
                                      in_=obj2[:])

    nc.compile()
    return nc


_NC_CACHE = None


def _get_program():
    global _NC_CACHE
    if _NC_CACHE is None:
        _NC_CACHE = build_program()
    return _NC_CACHE


def make_in_maps(x_cls, x_reg, cls_score, W_qkv_cls, W_qkv_reg):
    import ml_dtypes
    bf = ml_dtypes.bfloat16
    xt_cls = np.ascontiguousarray(x_cls[0].T).astype(bf)   # [C, N]
    xt_reg = np.ascontiguousarray(x_reg[0].T).astype(bf)
    cs25 = (SCALE * cls_score).reshape(1, N).astype(np.float32)
    in_maps = []
    for h in range(NCORES):
        rows = np.r_[h * HD:(h + 1) * HD,
                     C + h * HD:C + (h + 1) * HD,
                     2 * C + h * HD:2 * C + (h + 1) * HD]
        in_maps.append({
            "xt_cls": xt_cls,
            "xt_reg": xt_reg,
            "wt_cls": np.ascontiguousarray(W_qkv_cls[rows].T).astype(bf),
            "wt_reg": np.ascontiguousarray(W_qkv_reg[rows].T).astype(bf),
            "cs25": cs25,
        })
    return in_maps


def assemble(results):
    """results: list over cores of dicts of np arrays -> full outputs."""
    out_cls = np.empty((1, N, 2 * C), np.float32)
    out_reg = np.empty((1, N, 2 * C), np.float32)
    sim = np.empty((N, N), np.float32)
    obj = np.empty((N, N), np.float32)
    for h in range(NCORES):
        r = results[h]
        out_cls[0, :, h * HD:(h + 1) * HD] = r["avt_cls"].T
        out_cls[0, :, C + h * HD:C + (h + 1) * HD] = r["vt_cls"].T
        out_reg[0, :, h * HD:(h + 1) * HD] = r["avt_reg"].T
        out_reg[0, :, C + h * HD:C + (h + 1) * HD] = r["vt_reg"].T
        # device row order for rank h: for g in 0..3: rows g*512+h*64 .. +64
        ridx = np.concatenate([
            np.arange(g * GROWS + h * RPG, g * GROWS + (h + 1) * RPG)
            for g in range(NGRP)])
        sim[ridx] = r["sim_rows"]
        obj[ridx] = r["obj_rows"]
    return out_cls, out_reg, sim, obj


def kernel(x_cls, x_reg, cls_score, fg_score, W_qkv_cls, W_qkv_reg):
    x_cls = np.asarray(x_cls, np.float32)
    x_reg = np.asarray(x_reg, np.float32)
    cls_score = np.asarray(cls_score, np.float32)
    W_qkv_cls = np.asarray(W_qkv_cls, np.float32)
    W_qkv_reg = np.asarray(W_qkv_reg, np.float32)

    nc = _get_program()
    in_maps = make_in_maps(x_cls, x_reg, cls_score, W_qkv_cls, W_qkv_reg)
    res = run_bass_kernel_spmd(nc, in_maps, core_ids=list(range(NCORES)))
    return assemble(res.results)


# revision 34
# speedup vs baseline: 1.0846x; 1.0114x over previous
"""Trainium2 Bass kernel for nn_Attention_aware_msa (sparse_attention).

Sharding: one attention head per NeuronCore (8 heads / 8 cores), per the
tensor-parallel hint.  All cross-core traffic is done with collectives:

  - 4 chunked ReduceScatters produce each rank's row-slice of the
    head-summed attention (for sim_round2).
  - An AllGather of the L2-normalized v^T (bf16) gives every core the full
    stacked Vn^T [1024, 2048]; Sum_h vn_h vn_h^T == Vn Vn^T, so each core
    computes only its row-slice of the head-summed gram matrices (sim/obj
    masks) locally.
  - An AllToAll delivers Vn^T[:, my_rows] as the stationary operand for
    that row-slice without any rank-dependent addressing.

Compute layout: QKV projections contract C=1024 on the PE with bf16
operands (x and W are shipped as bf16; fp32 PSUM accumulation), producing
q/k/v directly in transposed [d=128, n] layout; scores and attn@v run with
f32r / bf16 operands at full PE rate.  Scores fold all scaling into the operands:
k' columns are pre-scaled by 25*cs[m]/|k_m| (cls) / 25/|k_m| (reg), and
the 1/|q_n| row factor rides the ACT exp's per-partition scale AP.  exp is
computed without row-max subtraction (|logits| <= 25, safe in f32).
attn = 0.5*(exp_c/rowsum_c + exp_r/rowsum_r) in bf16; attn @ v is computed
transposed (out[d, n]) with V-natural stationary tiles and PE-transposed
attn as the moving operand.

Host marshals x^T and per-head weight slices in, and reassembles the four
outputs (out_cls, out_reg, sim_round2, obj_mask) from per-core pieces.
"""
import numpy as np

try:
    import jax
    jax.config.update("jax_compilation_cache_dir", "/tmp/bass_jax_cache")
    jax.config.update("jax_persistent_cache_min_compile_time_secs", 0.0)
except Exception:
    pass

import concourse.bacc as bacc
import concourse.tile as tile
from concourse import mybir
from concourse.bass_utils import run_bass_kernel_spmd
from concourse.masks import make_identity

F32 = mybir.dt.float32
F32R = mybir.dt.float32r
BF16 = mybir.dt.bfloat16

B, N, C, H = 1, 2048, 1024, 8
HD = C // H            # 128 head dim
NCORES = 8
SCALE = 25.0
SIM_THRESH = 0.75
CONF_SIM_THRESH = 0.99
NCH = N // 128         # 16 row chunks of 128
NGRP = 4               # chunk groups (512 rows each) -> 4 ReduceScatters
GROWS = N // NGRP      # 512 rows per group
RPG = GROWS // NCORES  # 64 rows per rank per group


def build_program():
    nc = bacc.Bacc("TRN2", target_bir_lowering=False, debug=False,
                   num_devices=NCORES)

    # ---- parameters ----
    xt_cls = nc.declare_dram_parameter("xt_cls", [C, N], BF16, isOutput=False)
    xt_reg = nc.declare_dram_parameter("xt_reg", [C, N], BF16, isOutput=False)
    wt_cls = nc.declare_dram_parameter("wt_cls", [C, 3 * HD], BF16, isOutput=False)
    wt_reg = nc.declare_dram_parameter("wt_reg", [C, 3 * HD], BF16, isOutput=False)
    cs25 = nc.declare_dram_parameter("cs25", [1, N], F32, isOutput=False)

    avt_cls_o = nc.declare_dram_parameter("avt_cls", [HD, N], F32, isOutput=True)
    avt_reg_o = nc.declare_dram_parameter("avt_reg", [HD, N], F32, isOutput=True)
    vt_cls_o = nc.declare_dram_parameter("vt_cls", [HD, N], F32, isOutput=True)
    vt_reg_o = nc.declare_dram_parameter("vt_reg", [HD, N], F32, isOutput=True)
    sim_o = nc.declare_dram_parameter("sim_rows", [N // NCORES, N], F32, isOutput=True)
    obj_o = nc.declare_dram_parameter("obj_rows", [N // NCORES, N], F32, isOutput=True)

    rg = [list(range(NCORES))]
    AF = mybir.ActivationFunctionType
    OP = mybir.AluOpType
    X = mybir.AxisListType.X

    with tile.TileContext(nc) as tc:
        with (
            tc.tile_pool(name="pc", bufs=1) as pc,
            tc.tile_pool(name="dram", bufs=1, space="DRAM") as dram,
        ):
            # ---------- static tiles ----------
            ident_bf = pc.tile([128, 128], BF16, tag="ident_bf")
            make_identity(nc, ident_bf)
            ident_f32 = pc.tile([128, 128], F32, tag="ident_f32")
            make_identity(nc, ident_f32)
            ones_f = pc.tile([128, 1], F32, tag="ones_f")
            nc.vector.memset(ones_f, 1.0)
            ones_col = pc.tile([128, 1], F32R, tag="ones")
            nc.vector.tensor_copy(ones_col[:], ones_f[:])
            cs_t = pc.tile([1, N], F32, tag="cs")
            nc.sync.dma_start(out=cs_t, in_=cs25[:, :])

            # long-lived per-head tensors
            qt = {}    # f32r  q^T (unnormalized)
            kp = {}    # f32r  scaled k'^T
            vn = {}    # bf16  normalized v^T
            vnat = {}  # bf16  V natural, block j at [:, j*128:(j+1)*128]
            rq = {}    # [128, 16] f32 per-partition 1/|q|
            for nm in ("cls", "reg"):
                qt[nm] = pc.tile([128, N], F32R, tag=f"qt_{nm}", name=f"qt_{nm}")
                kp[nm] = pc.tile([128, N], F32R, tag=f"kp_{nm}", name=f"kp_{nm}")
                vn[nm] = pc.tile([128, N], BF16, tag=f"vn_{nm}", name=f"vn_{nm}")
                vnat[nm] = pc.tile([128, N], BF16, tag=f"vnat_{nm}", name=f"vnat_{nm}")
                rq[nm] = pc.tile([128, 16], F32, tag=f"rq_{nm}", name=f"rq_{nm}")

            rq_scratch = dram.tile([2, N], F32, name="rq_scratch")

            # ================= phase A+B: projections & norms =============
            with (
                tc.tile_pool(name="early", bufs=1) as early,
                tc.tile_pool(name="xt", bufs=3) as xtp,
                tc.tile_pool(name="sqp", bufs=2) as sqp,
                tc.tile_pool(name="ps_a", bufs=1, space="PSUM") as ps_a,
            ):
                for ii, nm in enumerate(("cls", "reg")):
                    xpar = xt_cls if nm == "cls" else xt_reg
                    wpar = wt_cls if nm == "cls" else wt_reg
                    wts = early.tile([128, 8, 3 * HD], BF16, tag=f"wt_{nm}",
                                     name=f"wt_{nm}")
                    nc.sync.dma_start(
                        out=wts, in_=wpar.ap().rearrange("(c8 p) d -> p c8 d",
                                                         p=128))
                    vt = early.tile([128, N], F32, tag=f"vt_{nm}", name=f"vt_{nm}")
                    ssq = {}
                    for t in ("q", "k", "v"):
                        ssq[t] = early.tile([1, N], F32, tag=f"ssq_{nm}_{t}",
                                            name=f"ssq_{nm}_{t}")
                    dst = {"q": qt[nm], "k": kp[nm], "v": vt}
                    for half in range(2):
                        pps = {}
                        for pi in range(3):
                            for nl in range(2):
                                pps[pi, nl] = ps_a.tile([128, 512], F32, tag=f"proj{pi}{nl}", name=f"pps{pi}{nl}")
                        for c8 in range(8):
                            xc = xtp.tile([128, 1024], BF16, tag="xchunk")
                            nc.sync.dma_start(
                                out=xc,
                                in_=xpar[c8 * 128:(c8 + 1) * 128,
                                         half * 1024:(half + 1) * 1024])
                            for nl in range(2):
                                for pi in range(3):
                                    nc.tensor.matmul(
                                        pps[pi, nl][:],
                                        wts[:, c8, pi * HD:(pi + 1) * HD],
                                        xc[:, nl * 512:(nl + 1) * 512],
                                        start=(c8 == 0), stop=(c8 == 7))
                        for pi, t in enumerate(("k", "q", "v")):
                            for nl in range(2):
                                n4 = half * 2 + nl
                                sl = slice(n4 * 512, (n4 + 1) * 512)
                                ppt = pps[("q", "k", "v").index(t), nl]
                                sq = sqp.tile([128, 512], F32R, tag=f"sq_{nm}")
                                nc.scalar.activation(out=sq[:], in_=ppt[:],
                                                     func=AF.Square)
                                nc.any.tensor_copy(out=dst[t][:, sl],
                                                   in_=ppt[:])
                                sps = ps_a.tile([1, 512], F32, tag="misc_ps", bufs=2)
                                nc.tensor.matmul(sps[:], ones_col[:], sq[:],
                                                 start=True, stop=True)
                                nc.any.tensor_copy(out=ssq[t][:, sl],
                                                   in_=sps[:])

                                if t == "k":
                                    # pipeline k' = k * (25*[cs]/|k|) per
                                    # 512-column slice as soon as its column
                                    # sums land -- kp readiness gates the
                                    # whole attention phase.
                                    bk4 = early.tile([1, 512], F32,
                                                     tag=f"bk4_{nm}",
                                                     name="bk4", bufs=2)
                                    nc.scalar.activation(out=bk4[:],
                                                         in_=ssq["k"][0:1, sl],
                                                         func=AF.Sqrt)
                                    nc.vector.reciprocal(bk4[:], bk4[:])
                                    if nm == "cls":
                                        nc.vector.tensor_tensor(
                                            out=bk4[:], in0=bk4[:],
                                            in1=cs_t[0:1, sl], op=OP.mult)
                                    else:
                                        nc.vector.tensor_scalar_mul(
                                            out=bk4[:], in0=bk4[:],
                                            scalar1=SCALE)
                                    bk_bc = early.tile([128, 512], F32,
                                                       tag="bc_big", name="bk_bc",
                                                       bufs=2)
                                    last_bcast = nc.gpsimd.partition_broadcast(
                                        bk_bc[:], bk4[:])
                                    nc.vector.tensor_tensor(
                                        out=kp[nm][:, sl],
                                        in0=kp[nm][:, sl].bitcast(F32),
                                        in1=bk_bc[:], op=OP.mult)

                    # ---------- norms for this input ----------

                    # vn = v / |v| (bf16)
                    rv = early.tile([1, N], F32, tag=f"rowc_{nm}", name="rv")
                    nc.scalar.activation(out=rv[:], in_=ssq["v"][:], func=AF.Sqrt)
                    nc.vector.reciprocal(rv[:], rv[:])
                    rv_bc = early.tile([128, N], F32, tag="bc_big", name="rv_bc", bufs=2)
                    last_bcast = nc.gpsimd.partition_broadcast(rv_bc[:], rv[:])
                    nc.vector.tensor_tensor(out=vn[nm][:], in0=vt[:],
                                            in1=rv_bc[:], op=OP.mult)

                    # rq: 1/|q_n| rearranged to partition-major [128, 16]
                    nc.sync.dma_start(out=rq_scratch[ii], in_=ssq["q"][0:1, :])
                    nc.sync.dma_start(
                        out=rq[nm],
                        in_=rq_scratch[ii].rearrange("(j p) -> p j", p=128))
                    nc.scalar.activation(out=rq[nm][:], in_=rq[nm][:], func=AF.Sqrt)
                    nc.vector.reciprocal(rq[nm][:], rq[nm][:])

                    # V natural (bf16): PE transpose of raw v^T
                    for j in range(NCH):
                        tps = ps_a.tile([128, 128], F32, tag="misc_ps", bufs=2)
                        nc.tensor.transpose(
                            tps[:], vt[:, j * 128:(j + 1) * 128], ident_f32[:])
                        nc.any.tensor_copy(
                            out=vnat[nm][:, j * 128:(j + 1) * 128], in_=tps[:])

                    # raw v^T is a kernel output
                    nc.sync.dma_start(
                        out=(vt_cls_o if nm == "cls" else vt_reg_o)[:, :],
                        in_=vt[:])

            # ---------- collectives: AllGather + AllToAll of vn ----------
            # Keep the Pool queue clear for phase-B broadcasts: every
            # collective waits on the last partition_broadcast so the Tile
            # scheduler cannot hoist a long AllGather ahead of them.
            from concourse.bass import _add_dep_helper
            NMI = {"cls": 0, "reg": 1}
            # one merged AllGather: rank block = [nm, 128, N]
            agi = dram.tile([2, 128, N], BF16, name="agi")
            for nm in ("cls", "reg"):
                nc.sync.dma_start(out=agi[NMI[nm]], in_=vn[nm][:])
            ag_out = dram.tile([NCORES, 2, 128, N], BF16, name="ago",
                               addr_space="Shared")
            cc = nc.gpsimd.collective_compute(
                "AllGather", OP.bypass, replica_groups=rg,
                ins=[agi[:].opt()], outs=[ag_out[:].opt()])
            _add_dep_helper(cc.ins, last_bcast.ins, sync=True,
                            reason="collectives after phase-B broadcasts")

            # one merged AllToAll: dest block j = both nm's columns
            # {g*512 + j*64 .. +64 : g in 0..3}
            a2i = dram.tile([NCORES, 2, 128, NGRP, RPG], BF16, name="a2i")
            for nm in ("cls", "reg"):
                for j in range(NCORES):
                    for g in range(NGRP):
                        c0 = g * GROWS + j * RPG
                        nc.sync.dma_start(out=a2i[j, NMI[nm], :, g, :],
                                          in_=vn[nm][:, c0:c0 + RPG])
            a2a_out = dram.tile([NCORES, 2, 128, NGRP, RPG], BF16, name="a2o")
            cc = nc.gpsimd.collective_compute(
                "AllToAll", OP.bypass, replica_groups=rg,
                ins=[a2i[:].opt()], outs=[a2a_out[:].opt()])
            _add_dep_helper(cc.ins, last_bcast.ins, sync=True,
                            reason="collectives after phase-B broadcasts")

            # ================= phase C: attention =========================
            att_bounce = [dram.tile([GROWS, N], BF16, name=f"attb_{g}") for g in range(NGRP)]
            rs_out = [dram.tile([RPG, N], BF16, name=f"rsout_{g}") for g in range(NGRP)]

            with (
                tc.tile_pool(name="cw", bufs=1) as cw,
                tc.tile_pool(name="attn", bufs=9) as attnp,
                tc.tile_pool(name="attnT", bufs=2) as attnTp,
                tc.tile_pool(name="avsb", bufs=2) as avsb,
                tc.tile_pool(name="ps_c", bufs=3, space="PSUM") as ps_c,
                tc.tile_pool(name="ps_t", bufs=1, space="PSUM") as ps_t,
                tc.tile_pool(name="ps_av", bufs=1, space="PSUM") as ps_av,
            ):
                attn_chunks = []
                atw = None
                for i in range(NCH):
                    g = i // 4
                    if i % 4 == 0:
                        atw = attnTp.tile([128, NCH * 512], BF16, tag="attnT",
                                          name=f"atw_{g}")
                    ec = {}
                    rec = {}
                    for nm in ("cls", "reg"):
                        e = cw.tile([128, N], BF16, tag=f"e_{nm}", name=f"e_{nm}", bufs=2)
                        parts = cw.tile([128, 2], F32, tag=f"parts_{nm}", name=f"parts_{nm}", bufs=3)
                        for mh in range(2):
                            # two-bank score psum: 2 matmuls, ONE exp over
                            # [128, 1024] (halves the per-chunk hop count)
                            sps = ps_c.tile([128, 1024], F32, tag="score")
                            for mq in range(2):
                                m4 = mh * 2 + mq
                                nc.tensor.matmul(
                                    sps[:, mq * 512:(mq + 1) * 512],
                                    qt[nm][:, i * 128:(i + 1) * 128],
                                    kp[nm][:, m4 * 512:(m4 + 1) * 512],
                                    start=True, stop=True)
                            nc.scalar.activation(
                                out=e[:, mh * 1024:(mh + 1) * 1024], in_=sps[:],
                                func=AF.Exp, scale=rq[nm][:, i:i + 1],
                                accum_out=parts[:, mh:mh + 1])
                        rs_sum = cw.tile([128, 1], F32, tag=f"rssum_{nm}", name=f"rssum_{nm}", bufs=3)
                        nc.vector.reduce_sum(out=rs_sum[:], in_=parts[:], axis=X)
                        rc = cw.tile([128, 1], F32, tag=f"rec_{nm}", name=f"rec_{nm}", bufs=3)
                        nc.vector.reciprocal(rc[:], rs_sum[:])
                        ec[nm] = e
                        rec[nm] = rc

                    t1 = cw.tile([128, N], BF16, tag="t1")
                    nc.vector.tensor_scalar(
                        out=t1[:], in0=ec["cls"][:], scalar1=rec["cls"][:],
                        scalar2=0.5, op0=OP.mult, op1=OP.mult)
                    t2 = cw.tile([128, N], BF16, tag="t2")
                    nc.vector.tensor_scalar(
                        out=t2[:], in0=ec["reg"][:], scalar1=rec["reg"][:],
                        scalar2=0.5, op0=OP.mult, op1=OP.mult)
                    ab = attnp.tile([128, N], BF16, tag="attn_bf")
                    nc.vector.tensor_tensor(out=ab[:], in0=t1[:], in1=t2[:],
                                            op=OP.add)
                    attn_chunks.append(ab)
                    nc.sync.dma_start(
                        out=att_bounce[g][(i % 4) * 128:(i % 4 + 1) * 128, :],
                        in_=ab[:])

                    if i % 4 == 3:
                        # ReduceScatter for this group of 512 rows
                        nc.gpsimd.collective_compute(
                            "ReduceScatter", OP.add, replica_groups=rg,
                            ins=[att_bounce[g][:].opt()],
                            outs=[rs_out[g][:].opt()])

                        # transpose the 4 chunks -> attnT window [m, 512]
                        for j in range(NCH):
                            tps = ps_t.tile([128, 512], BF16, tag="attr")
                            for ii in range(4):
                                nc.tensor.transpose(
                                    tps[:, ii * 128:(ii + 1) * 128],
                                    attn_chunks[g * 4 + ii][:, j * 128:(j + 1) * 128],
                                    ident_bf[:])
                            nc.any.tensor_copy(
                                out=atw[:, j * 512:(j + 1) * 512], in_=tps[:])

                        # AV for these 512 output columns
                        for nm, opar in (("cls", avt_cls_o), ("reg", avt_reg_o)):
                            aps = ps_av.tile([128, 512], F32, tag="av")
                            for j in range(NCH):
                                nc.tensor.matmul(
                                    aps[:],
                                    vnat[nm][:, j * 128:(j + 1) * 128],
                                    atw[:, j * 512:(j + 1) * 512],
                                    start=(j == 0), stop=(j == NCH - 1))
                            av_s = avsb.tile([128, 512], F32, tag="av_sb")
                            nc.any.tensor_copy(out=av_s[:], in_=aps[:])
                            nc.sync.dma_start(
                                out=opar[:, g * 512:(g + 1) * 512], in_=av_s[:])

            # ================= phase D: gram row-slices + masks ===========
            with (
                tc.tile_pool(name="late", bufs=1) as late,
                tc.tile_pool(name="vgp", bufs=3) as vgp,
                tc.tile_pool(name="pe_big", bufs=3) as peb,
                tc.tile_pool(name="ps_g", bufs=1, space="PSUM") as ps_g,
            ):
                masks = {}
                for nm, thr in (("cls", SIM_THRESH * H), ("reg", CONF_SIM_THRESH * H)):
                    a2s = late.tile([128, NCORES * NGRP * RPG], BF16, tag=f"a2s_{nm}", name=f"a2s_{nm}")
                    nc.sync.dma_start(
                        out=a2s[:].rearrange("p (h g r) -> p h g r",
                                             h=NCORES, g=NGRP),
                        in_=a2a_out[:, NMI[nm]].rearrange("h p g r -> p h g r"))
                    mask = late.tile([128, 2, N], BF16, tag=f"mask_{nm}", name=f"mask_{nm}")
                    gt = {}
                    for mm in range(2):
                        for m4 in range(4):
                            gt[mm, m4] = ps_g.tile([128, 512], F32, tag=f"gram{mm}{m4}", name=f"gram{mm}{m4}")
                    for c8 in range(8):
                        vg = vgp.tile([128, N], BF16, tag="vg")
                        nc.sync.dma_start(out=vg, in_=ag_out[c8, NMI[nm]])
                        for mm in range(2):
                            for m4 in range(4):
                                nc.tensor.matmul(
                                    gt[mm, m4][:],
                                    a2s[:, c8 * 256 + mm * 128:
                                        c8 * 256 + (mm + 1) * 128],
                                    vg[:, m4 * 512:(m4 + 1) * 512],
                                    start=(c8 == 0), stop=(c8 == 7))
                    for mm in range(2):
                        for m4 in range(4):
                            nc.vector.tensor_scalar(
                                out=mask[:, mm, m4 * 512:(m4 + 1) * 512],
                                in0=gt[mm, m4][:], scalar1=float(thr),
                                scalar2=None, op0=OP.is_gt)
                    masks[nm] = mask

                # ---------- phase E: sim_round2 / obj_mask rows ----------
                for mm in range(2):
                    srows = late.tile([128, N], BF16, tag="srows")
                    for gg in range(2):
                        g = mm * 2 + gg
                        nc.sync.dma_start(out=srows[gg * RPG:(gg + 1) * RPG, :],
                                          in_=rs_out[g][:, :])
                    es = peb.tile([128, N], BF16, tag="pe_big")
                    essum = late.tile([128, 1], F32, tag="essum")
                    nc.scalar.activation(out=es[:], in_=srows[:], func=AF.Exp,
                                         scale=1.0 / H, accum_out=essum[:])
                    # the softmax denominator AND the sim renormalizer both
                    # cancel in the masked renorms:
                    #   sim2 = es*mask_cls / sum(es*mask_cls)
                    #   obj  = es*mask_cls*mask_reg / sum(es*mask_cls*mask_reg)
                    msk = peb.tile([128, N], BF16, tag="pe_big")
                    nc.vector.tensor_tensor(out=msk[:], in0=es[:],
                                            in1=masks["cls"][:, mm, :], op=OP.mult)
                    mrow = late.tile([128, 1], F32, tag="mrow")
                    nc.vector.reduce_sum(out=mrow[:], in_=msk[:], axis=X)
                    rmrow = late.tile([128, 1], F32, tag="rmrow")
                    nc.vector.reciprocal(rmrow[:], mrow[:])
                    sim2 = peb.tile([128, N], F32, tag="pe_big")
                    nc.vector.tensor_scalar(out=sim2[:], in0=msk[:],
                                            scalar1=rmrow[:], scalar2=None,
                                            op0=OP.mult)
                    nc.sync.dma_start(out=sim_o[mm * 128:(mm + 1) * 128, :],
                                      in_=sim2[:])

                    omsk = peb.tile([128, N], BF16, tag="pe_big")
                    nc.vector.tensor_tensor(out=omsk[:], in0=msk[:],
                                            in1=masks["reg"][:, mm, :], op=OP.mult)
                    orow = late.tile([128, 1], F32, tag="orow")
                    nc.vector.reduce_sum(out=orow[:], in_=omsk[:], axis=X)
                    rorow = late.tile([128, 1], F32, tag="rorow")
                    nc.vector.reciprocal(rorow[:], orow[:])
                    obj2 = peb.tile([128, N], F32, tag="pe_big")
                    nc.vector.tensor_scalar(out=obj2[:], in0=omsk[:],
                                            scalar1=rorow[:], scalar2=None,
                                            op0=OP.mult)
                    nc.sync.dma_start(out=obj_o[mm * 128:(mm + 1) * 128, :],
                                      in_=obj2[:])

    nc.compile()
    return nc


_NC_CACHE = None


def _get_program():
    global _NC_CACHE
    if _NC_CACHE is None:
        _NC_CACHE = build_program()
    return _NC_CACHE


def make_in_maps(x_cls, x_reg, cls_score, W_qkv_cls, W_qkv_reg):
    import ml_dtypes
    bf = ml_dtypes.bfloat16
    xt_cls = np.ascontiguousarray(x_cls[0].T).astype(bf)   # [C, N]
    xt_reg = np.ascontiguousarray(x_reg[0].T).astype(bf)
    cs25 = (SCALE * cls_score).reshape(1, N).astype(np.float32)
    in_maps = []
    for h in range(NCORES):
        rows = np.r_[h * HD:(h + 1) * HD,
                     C + h * HD:C + (h + 1) * HD,
                     2 * C + h * HD:2 * C + (h + 1) * HD]
        in_maps.append({
            "xt_cls": xt_cls,
            "xt_reg": xt_reg,
            "wt_cls": np.ascontiguousarray(W_qkv_cls[rows].T).astype(bf),
            "wt_reg": np.ascontiguousarray(W_qkv_reg[rows].T).astype(bf),
            "cs25": cs25,
        })
    return in_maps


def assemble(results):
    """results: list over cores of dicts of np arrays -> full outputs."""
    out_cls = np.empty((1, N, 2 * C), np.float32)
    out_reg = np.empty((1, N, 2 * C), np.float32)
    sim = np.empty((N, N), np.float32)
    obj = np.empty((N, N), np.float32)
    for h in range(NCORES):
        r = results[h]
        out_cls[0, :, h * HD:(h + 1) * HD] = r["avt_cls"].T
        out_cls[0, :, C + h * HD:C + (h + 1) * HD] = r["vt_cls"].T
        out_reg[0, :, h * HD:(h + 1) * HD] = r["avt_reg"].T
        out_reg[0, :, C + h * HD:C + (h + 1) * HD] = r["vt_reg"].T
        # device row order for rank h: for g in 0..3: rows g*512+h*64 .. +64
        ridx = np.concatenate([
            np.arange(g * GROWS + h * RPG, g * GROWS + (h + 1) * RPG)
            for g in range(NGRP)])
        sim[ridx] = r["sim_rows"]
        obj[ridx] = r["obj_rows"]
    return out_cls, out_reg, sim, obj


def kernel(x_cls, x_reg, cls_score, fg_score, W_qkv_cls, W_qkv_reg):
    x_cls = np.asarray(x_cls, np.float32)
    x_reg = np.asarray(x_reg, np.float32)
    cls_score = np.asarray(cls_score, np.float32)
    W_qkv_cls = np.asarray(W_qkv_cls, np.float32)
    W_qkv_reg = np.asarray(W_qkv_reg, np.float32)

    nc = _get_program()
    in_maps = make_in_maps(x_cls, x_reg, cls_score, W_qkv_cls, W_qkv_reg)
    res = run_bass_kernel_spmd(nc, in_maps, core_ids=list(range(NCORES)))
    return assemble(res.results)
